# revision 11
# baseline (speedup 1.0000x reference)
"""Head-parallel dual-branch attention kernel for one TRN2 chip (8 cores).

Sharding: core (b, hh) = batch b (0-3) x head-half hh (0-1).  Each core
computes BOTH branches for its 8 heads and emits per-branch yT-partials
(proj row-sharded over channels, f32); the host sums the two head-half
partials and adds the bias during unshard.  Zero device comm.

Key structure (per core, SPMD-uniform):
  A1: qk [1024, N] = [q|k]-half weights^T @ x^T, via fp8 hi+lo residual
      pairs and DoubleRow matmuls (3 cross terms per 256-chunk = 0.75x
      the bf16 PE cost at ~bf16 accuracy).
  A2: vn [N, 8*65] = x @ v-half^T (+ ones col per head) same hi/lo trick;
      vT per head-pair via SBUF->SBUF DMA transpose of vn (bf16).
  per unit (br, h) of 16 (br0 units first, then br1):
    S[m-block, n] = lhsT(kT|vT) x rhs(qT|vT), K=64, bf16   (br1: upper
      blocks only; symmetric part mirrored in est domain via DMA
      transpose - est is symmetric for (v,v) before masking)
    est = exp(S) on Act (activation, scale=1) or DVE/Pool
      (tensor_tensor pow: e_tile ** S) - exp work is split across all
      three elementwise engines.
    est *= em[br] (exp of mask, DVE/Pool)
    PV[n, 8x65] += est(m)^T x vn(m) (65th col = ones -> denominator)
    OT[n, 512] = PV[:, :64] * recip(PV[:, 64])  per head
    ot[c, n] via DMA transpose of OT (no PE transpose)
  proj: yT_br[co, n] += pwT^T @ ot, bf16; psum -> DRAM f32 directly.

Host adds proj bias and sums head-half partials in f32.
"""

import numpy as np
import ml_dtypes

import concourse.bass as bass
from concourse import bacc
import concourse.tile as tile
import concourse.mybir as mybir
from contextlib import ExitStack

B, N, C, H, D, P = 4, 1024, 1024, 16, 64, 128
HH = 8          # heads per core
NHALF = 512
BF16 = mybir.dt.bfloat16
F32 = mybir.dt.float32
FP8 = mybir.dt.float8e4
AF = mybir.ActivationFunctionType
PM = mybir.MatmulPerfMode
ALU = mybir.AluOpType

# psum scores arrive scaled by LAM = lam_q*lam_k (power-folded into the fp8
# hi/lo weights to stay in e4m3's normal range); exp undoes it via
# activation-scale / pow-base.
LAM = 2048.0
GAMMA = 1.0 / LAM

_nc_cache = None

# engine split tables (tuned against the cost-model trace)
# exp engine per (br, u*8+m); mul engine per (br, u*8+m)
# walrus: GPSIMD cannot access PSUM -> Pool only gets SBUF-side est*em muls;
# exps (psum reads) split Act/DVE, copies on Act, otu/rcp on DVE.
# walrus codegen rejects ALU pow on DVE too -> all exps via Act activation.
EXP0 = ["A"] * 8
EXP1 = ["A"] * 8
MUL0 = ["P", "P", "P", "V", "P", "P", "P", "V"]
MUL1 = ["P", "P", "P", "V", "P", "P", "P", "V"]


def _build(reps=1):
    nc = bacc.Bacc("TRN2", target_bir_lowering=False, debug=False, num_devices=8)
    x8 = nc.declare_dram_parameter("x8", [2 * 4 * 256, N], FP8, isOutput=False)
    w8 = nc.declare_dram_parameter("w8", [2 * 4 * 256, 3 * NHALF], FP8, isOutput=False)
    em0 = nc.declare_dram_parameter("em0", [N, N], BF16, isOutput=False)
    em1 = nc.declare_dram_parameter("em1", [N, N], BF16, isOutput=False)
    pwT = nc.declare_dram_parameter("pwT", [NHALF, C], BF16, isOutput=False)
    out0 = nc.declare_dram_parameter("out0", [C, N], BF16, isOutput=True)
    out1 = nc.declare_dram_parameter("out1", [C, N], BF16, isOutput=True)

    with tile.TileContext(nc) as tc:
        for _ in range(reps):
            with ExitStack() as ctx:
                _body(tc, ctx, x8, w8, em0, em1, pwT, out0, out1)
    nc.compile()
    return nc


def _body(tc, ctx, x8, w8, em0, em1, pwT, out0, out1):
    nc = tc.nc

    pers = ctx.enter_context(tc.tile_pool(name="pers", bufs=1))
    work = ctx.enter_context(tc.tile_pool(name="work", bufs=1))
    psum = ctx.enter_context(tc.tile_pool(name="psum", bufs=1, space="PSUM"))

    EW = {"A": nc.scalar, "V": nc.vector, "P": nc.gpsimd}

    def ew_copy(e, dst, src):
        if e == "A":
            nc.scalar.copy(dst, src)
        else:
            EW[e].tensor_copy(dst, src)

    # ---------------- persistent tiles ----------------
    # x8t[hl][g], w8t[hl][g]: fp8 hi/lo chunk-pair tiles for DoubleRow
    x8t = [[pers.tile([P, 2, N], FP8, name=f"x{hl}{g}", tag=f"x{hl}{g}")
            for g in range(4)] for hl in range(2)]
    w8t = [[pers.tile([P, 2, 3 * NHALF], FP8, name=f"w{hl}{g}", tag=f"w{hl}{g}")
            for g in range(4)] for hl in range(2)]
    qk_t = [pers.tile([P, N], BF16, name=f"qk{i}", tag=f"qk{i}") for i in range(8)]
    vd = pers.tile([P, 8, NHALF], BF16, name="vd", tag="vd")
    ones = pers.tile([P, 1], BF16, name="ones", tag="ones")
    vT = [pers.tile([P, N], BF16, name=f"vT{g}", tag=f"vT{g}") for g in range(4)]
    em_t = [[pers.tile([P, N], BF16, name=f"em{br}_{m}", tag=f"em{br}_{m}")
             for m in range(8)] for br in range(2)]
    Bt = pers.tile([P, N], F32, name="Bt", tag="Bt")
    ot_t = [pers.tile([P, N], BF16, name=f"ot{i}", tag=f"ot{i}") for i in range(8)]
    pw_t = [pers.tile([P, C], BF16, name=f"pw{g}", tag=f"pw{g}") for g in range(4)]

    def pts_tile(ui):
        return work.tile([P, 8, N], BF16, name=f"pts{ui}", tag="pts", bufs=3)

    def st_ps(nm):
        return psum.tile([P, N], F32, name=nm, tag="st", bufs=2)

    def pv_ps(nm):
        return psum.tile([P, 2, NHALF], F32, name=nm, tag="pv", bufs=2)

    # ---------------- input DMA staging ----------------
    # g0 first so A1 pair-0 can start immediately; issue on SP + helpers.
    def dma_x(hl, g, eng=nc.sync):
        src = x8[(hl * 4 + g) * 256:(hl * 4 + g + 1) * 256, :]
        eng.dma_start(x8t[hl][g][:], src.rearrange("(p two) n -> p two n", two=2))

    def dma_w(hl, g, eng=nc.sync):
        src = w8[(hl * 4 + g) * 256:(hl * 4 + g + 1) * 256, :]
        eng.dma_start(w8t[hl][g][:], src.rearrange("(p two) n -> p two n", two=2))

    def dma_em(br, m, eng=nc.sync):
        src = em0 if br == 0 else em1
        eng.dma_start(em_t[br][m][:], src[m * P:(m + 1) * P, :])

    for g in range(2):
        dma_w(0, g, nc.sync)
        dma_x(0, g, nc.scalar)
        dma_w(1, g, nc.gpsimd)
        dma_x(1, g, nc.sync)
    nc.vector.memset(Bt[:], float(np.exp(GAMMA)))
    nc.vector.memset(ones[:], 1.0)
    for g in range(2, 4):
        dma_w(0, g, nc.sync)
        dma_x(0, g, nc.scalar)
        dma_w(1, g, nc.gpsimd)
        dma_x(1, g, nc.sync)
    for m in range(8):
        dma_em(0, m, nc.sync if m % 2 == 0 else nc.gpsimd)

    # ---------------- fill closures ----------------
    def hilo_mms(ps_half, lhs_of, rhs_of, first, last):
        """3-cross-term hi/lo fp8 DR accumulation over 4 chunk-pairs."""
        for g in range(4):
            combos = ((0, 0), (1, 0), (0, 1))
            for ci, (lh, rh) in enumerate(combos):
                nc.tensor.matmul(
                    ps_half,
                    lhsT=lhs_of(lh, g), rhs=rhs_of(rh, g),
                    start=(first and g == 0 and ci == 0),
                    stop=(last and g == 3 and ci == 2),
                    perf_mode=PM.DoubleRow,
                )

    def a1_fill(cc, nh, ceng):
        ps = st_ps(f"a1_{cc}_{nh}")
        half = ps[:, nh * NHALF:(nh + 1) * NHALF]
        hilo_mms(
            half,
            lambda hl, g: w8t[hl][g][:, :, cc * P:(cc + 1) * P],
            lambda hl, g: x8t[hl][g][:, :, nh * NHALF:(nh + 1) * NHALF],
            True, True,
        )
        ew_copy(ceng, qk_t[cc][:, nh * NHALF:(nh + 1) * NHALF], half)

    def a2_fill(m, ceng):
        ps = st_ps(f"a2_{m}")
        half = ps[:, 0:NHALF]
        hilo_mms(
            half,
            lambda hl, g: x8t[hl][g][:, :, m * P:(m + 1) * P],
            lambda hl, g: w8t[hl][g][:, :, 2 * NHALF:3 * NHALF],
            True, True,
        )
        ew_copy(ceng, vd[:, m, :], half)
        for g in range(4):
            nc.sync.dma_start(
                vT[g][:, m * P:(m + 1) * P],
                vd[:, m, g * P:(g + 1) * P],
                transpose=True,
            )

    def proj_fill(br, mt):
        ps = st_ps(f"y{br}_{mt}")
        out = out0 if br == 0 else out1
        for nh in range(2):
            for g in range(4):
                nc.tensor.matmul(
                    ps[:, nh * NHALF:(nh + 1) * NHALF],
                    lhsT=pw_t[g][:, mt * P:(mt + 1) * P],
                    rhs=ot_t[br * 4 + g][:, nh * NHALF:(nh + 1) * NHALF],
                    start=(g == 0), stop=(g == 3),
                )
        y = work.tile([P, N], BF16, name="y", tag="y", bufs=2)
        for nh in range(2):
            sl = slice(nh * NHALF, (nh + 1) * NHALF)
            ew_copy("V", y[:, sl], ps[:, sl])
            nc.sync.dma_start(out[mt * P:(mt + 1) * P, sl], y[:, sl])

    # Deadline-paced fillers (gstep -> closure)
    sched = {}
    cp_cyc = ["V", "V", "V", "V", "V", "V", "V", "V"]
    for m in range(8):
        sched[2 * m + 1] = (lambda m=m, e=cp_cyc[m]: a2_fill(m, e))
    gs = {1: [2, 6, 10, 14], 2: [18, 22, 26, 30], 3: [34, 38, 42, 46]}
    for pair in (1, 2, 3):
        fills = [(cc, nh) for nh in range(2) for cc in (pair, 4 + pair)]
        for g, (cc, nh) in zip(gs[pair], fills):
            e = cp_cyc[(cc + nh) % 8]
            sched[g] = (lambda cc=cc, nh=nh, e=e: a1_fill(cc, nh, e))

    def late_dmas():
        for m in range(8):
            dma_em(1, m, nc.sync)
        for g in range(4):
            nc.sync.dma_start(pw_t[g][:], pwT[g * P:(g + 1) * P, :])
    sched[17] = late_dmas

    # upfront A1 for head-pair 0
    for nh in range(2):
        for cc in (0, 4):
            a1_fill(cc, nh, cp_cyc[(cc + nh) % 8])

    # ---------------- attention units ----------------
    UNITS = [(0, h) for h in range(8)] + [(1, h) for h in range(8)]
    pts = {}

    def emit_pv(u_state, m):
        u, br, h, pv = u_state
        est = pts[u % 3]
        pv4 = pv[:, :, 0:260].rearrange("p b (j c) -> p b j c", c=65)
        for j in range(8):
            lhsT = est[:, m, j * P:(j + 1) * P]
            nc.tensor.matmul(
                pv4[:, j // 4, j % 4, 0:64],
                lhsT=lhsT, rhs=vd[:, m, 64 * h:64 * h + 64],
                start=(m == 0 and j % 4 == 0), stop=False,
            )
            nc.tensor.matmul(
                pv4[:, j // 4, j % 4, 64:65],
                lhsT=lhsT, rhs=ones[:],
                start=False, stop=(m == 7 and j % 4 == 3),
            )

    otu_pair = [None]

    def finish_unit(u_state):
        u, br, h, pv = u_state
        g = br * 4 + h // 2
        pv4 = pv[:, :, 0:260].rearrange("p b (j c) -> p b j c", c=65)
        rcp = work.tile([P, 2, 4, 1], F32, name="rcp", tag="rcp", bufs=2)
        nc.vector.reciprocal(rcp[:], pv4[:, :, :, 64:65])
        if h % 2 == 0:
            otu_pair[0] = work.tile([P, 2, 4, 2, 64], BF16, name=f"otu{u}",
                                    tag="otu", bufs=2)
        otu = otu_pair[0]
        nc.vector.tensor_mul(
            otu[:, :, :, h % 2, :], pv4[:, :, :, 0:64],
            rcp[:].broadcast_to((P, 2, 4, 64)))
        if h % 2 == 1:
            of = otu[:].rearrange("p b j two c -> p (b j) (two c)")
            for j in range(8):
                nc.sync.dma_start(
                    ot_t[g][:, j * P:(j + 1) * P],
                    of[:, j, :],
                    transpose=True,
                )

    proj_q = []
    gstep = 0
    hist = []
    for ui, (br, h) in enumerate(UNITS):
        if br == 0:
            kT, qT, ro = qk_t[4 + h // 2], qk_t[h // 2], (h % 2) * 64
        else:
            kT = qT = vT[h // 2]
            ro = (h % 2) * 64
        pv = pv_ps(f"pv{ui}")
        cur = (ui, br, h, pv)
        est = pts_tile(ui)
        pts[ui % 3] = est
        for m in range(8):
            if br == 0:
                ps = st_ps(f"st{ui}_{m}")
                for nh in range(2):
                    nc.tensor.matmul(
                        ps[:, nh * NHALF:(nh + 1) * NHALF],
                        lhsT=kT[ro:ro + 64, m * P:(m + 1) * P],
                        rhs=qT[ro:ro + 64, nh * NHALF:(nh + 1) * NHALF],
                        start=True, stop=True,
                    )
                e = EXP0[m]
                if e == "A":
                    nc.scalar.activation(est[:, m, :], ps[:], AF.Exp, scale=GAMMA)
                else:
                    EW[e].tensor_tensor(est[:, m, :], Bt[:], ps[:], ALU.pow)
                me = MUL0[m]
                EW[me].tensor_mul(est[:, m, :], est[:, m, :], em_t[0][m][:])
            else:
                mi = m
                ps = st_ps(f"st{ui}_{m}")
                if mi < 4:
                    nc.tensor.matmul(
                        ps[:, mi * P:NHALF],
                        lhsT=kT[ro:ro + 64, mi * P:(mi + 1) * P],
                        rhs=qT[ro:ro + 64, mi * P:NHALF],
                        start=True, stop=True,
                    )
                    nc.tensor.matmul(
                        ps[:, NHALF:N],
                        lhsT=kT[ro:ro + 64, mi * P:(mi + 1) * P],
                        rhs=qT[ro:ro + 64, NHALF:N],
                        start=True, stop=True,
                    )
                else:
                    nc.tensor.matmul(
                        ps[:, mi * P:N],
                        lhsT=kT[ro:ro + 64, mi * P:(mi + 1) * P],
                        rhs=qT[ro:ro + 64, mi * P:N],
                        start=True, stop=True,
                    )
                e = EXP1[m]
                if e == "A":
                    nc.scalar.activation(est[:, mi, mi * P:N], ps[:, mi * P:N],
                                         AF.Exp, scale=GAMMA)
                else:
                    EW[e].tensor_tensor(est[:, mi, mi * P:N], Bt[:, mi * P:N],
                                        ps[:, mi * P:N], ALU.pow)
                for k in range(mi):
                    nc.sync.dma_start(
                        est[:, mi, k * P:(k + 1) * P],
                        est[:, k, mi * P:(mi + 1) * P],
                        transpose=True,
                    )
            if len(hist) >= 2:
                emit_pv(hist[-2], m)
            if gstep in sched:
                sched.pop(gstep)()
            if proj_q and gstep % 3 == 0:
                proj_q.pop()()
            gstep += 1
        if br == 1:
            # est*em only after all mirrors read the pure-exp tiles
            for m in range(8):
                EW[MUL1[m]].tensor_mul(est[:, m, :], est[:, m, :], em_t[1][m][:])
        hist.append(cur)
        if len(hist) >= 3:
            finish_unit(hist[-3])
            if hist[-3][0] == 7:
                proj_q = [(lambda mt=mt: proj_fill(0, mt)) for mt in range(8)]
                proj_q.reverse()

    for u_state in hist[-2:]:
        for m in range(8):
            emit_pv(u_state, m)
        finish_unit(u_state)
        while proj_q:
            proj_q.pop()()
    for g in sorted(sched):
        sched.pop(g)()

    # ---------------- branch-1 projection (tail) ----------------
    for mt in range(8):
        proj_fill(1, mt)


def _hilo(a):
    """Split f32 array into fp8_e4m3 hi + lo residual."""
    hi = a.astype(ml_dtypes.float8_e4m3)
    lo = (a - hi.astype(np.float32)).astype(ml_dtypes.float8_e4m3)
    return hi, lo


def _pack_pairs(a):
    """[1024, cols] -> [4, 128, 2, cols] chunk-pair layout, flattened to
    [2048?, cols] rows ((g*128+p)*2+i)."""
    ch = a.reshape(4, 2, 128, -1).transpose(0, 2, 1, 3)  # g, p, i, cols
    return np.ascontiguousarray(ch.reshape(4 * 128 * 2, -1))


def _prep_inputs(x, attn_mask, qkv_w, proj_w, proj_b):
    """8 per-core input maps: core (b, hh) = batch b, head-half hh."""
    bf = ml_dtypes.bfloat16
    q_w, k_w, v_w = qkv_w[0:C], qkv_w[C:2 * C], qkv_w[2 * C:3 * C]
    s = float(D ** (-0.5))
    em0 = np.ascontiguousarray(np.exp(attn_mask[0, 0]).T.astype(bf))
    em1 = np.ascontiguousarray(np.exp(attn_mask[1, 0]).T.astype(bf))
    # scale folding: q' = 64*s*q, k' = 32*k, v' = sqrt(2048*s)*v so that
    # q'k' = 2048*s*qk and v'v' = 2048*s*vv (GAMMA=1/2048 undone in exp),
    # while keeping the fp8 hi/lo weights in e4m3's normal range.
    # lam_v*sqrt(s) = 16 exactly, compensated in pwT.
    lam_q, lam_k = 64.0, 32.0
    lam_v = float(np.sqrt(2048.0 * s))
    in_maps = []
    xp = {}
    for core in range(8):
        b, hh = core // 2, core % 2
        sl = slice(hh * NHALF, (hh + 1) * NHALF)
        if b not in xp:
            xT = np.ascontiguousarray(x[b].T.astype(np.float32))
            xh, xl = _hilo(xT)
            xp[b] = np.vstack([_pack_pairs(xh), _pack_pairs(xl)])
        wcat = np.hstack([(q_w[sl] * (s * lam_q)).T, (k_w[sl] * lam_k).T,
                          (v_w[sl] * lam_v).T]).astype(np.float32)
        wh, wl = _hilo(wcat)
        w8 = np.vstack([_pack_pairs(wh), _pack_pairs(wl)])
        pwTc = np.ascontiguousarray((proj_w[:, sl] / 16.0).T.astype(bf))
        in_maps.append({
            "x8": xp[b], "w8": w8, "em0": em0, "em1": em1, "pwT": pwTc,
        })
    return in_maps


def _run(inputs, trace=False, **kw):
    global _nc_cache
    from concourse.bass_utils import run_bass_kernel_spmd
    if _nc_cache is None:
        _nc_cache = _build()
    in_maps = _prep_inputs(**inputs)
    res = run_bass_kernel_spmd(_nc_cache, in_maps, core_ids=list(range(8)),
                               trace=trace, **kw)
    pb = np.asarray(inputs["proj_b"], dtype=np.float32)
    outs = []
    for br in range(2):
        nm = f"out{br}"
        ys = []
        for b in range(B):
            p0 = np.asarray(res.results[2 * b][nm], dtype=np.float32)
            p1 = np.asarray(res.results[2 * b + 1][nm], dtype=np.float32)
            ys.append((p0 + p1).T + pb)
        outs.append(np.stack(ys))
    x_ori, x_v = outs[0], outs[1]
    return (x_v, x_ori), res


def kernel(x, attn_mask, qkv_w, proj_w, proj_b):
    (x_v, x_ori), _ = _run(dict(x=np.asarray(x), attn_mask=np.asarray(attn_mask),
                                qkv_w=np.asarray(qkv_w), proj_w=np.asarray(proj_w),
                                proj_b=np.asarray(proj_b)))
    return (x_v, x_ori)


# revision 16
# speedup vs baseline: 1.0362x; 1.0362x over previous
"""Head-parallel dual-branch attention kernel for one TRN2 chip (8 cores).

Sharding: core (b, hh) = batch b (0-3) x head-half hh (0-1).  Each core
computes BOTH branches for its 8 heads and emits per-branch yT-partials
(proj row-sharded over channels, f32); the host sums the two head-half
partials and adds the bias during unshard.  Zero device comm.

Key structure (per core, SPMD-uniform):
  A1: qk [1024, N] = [q|k]-half weights^T @ x^T, via fp8 hi+lo residual
      pairs and DoubleRow matmuls (3 cross terms per 256-chunk = 0.75x
      the bf16 PE cost at ~bf16 accuracy).
  A2: vn [N, 8*65] = x @ v-half^T (+ ones col per head) same hi/lo trick;
      vT per head-pair via SBUF->SBUF DMA transpose of vn (bf16).
  per unit (br, h) of 16 (br0 units first, then br1):
    S[m-block, n] = lhsT(kT|vT) x rhs(qT|vT), K=64, bf16   (br1: upper
      blocks only; symmetric part mirrored in est domain via DMA
      transpose - est is symmetric for (v,v) before masking)
    est = exp(S) on Act (activation, scale=1) or DVE/Pool
      (tensor_tensor pow: e_tile ** S) - exp work is split across all
      three elementwise engines.
    est *= em[br] (exp of mask, DVE/Pool)
    PV[n, 8x65] += est(m)^T x vn(m) (65th col = ones -> denominator)
    OT[n, 512] = PV[:, :64] * recip(PV[:, 64])  per head
    ot[c, n] via DMA transpose of OT (no PE transpose)
  proj: yT_br[co, n] += pwT^T @ ot, bf16; psum -> DRAM f32 directly.

Host adds proj bias and sums head-half partials in f32.
"""

import numpy as np
import ml_dtypes

import concourse.bass as bass
from concourse import bacc
import concourse.tile as tile
import concourse.mybir as mybir
from contextlib import ExitStack

B, N, C, H, D, P = 4, 1024, 1024, 16, 64, 128
HH = 8          # heads per core
NHALF = 512
BF16 = mybir.dt.bfloat16
F32 = mybir.dt.float32
FP8 = mybir.dt.float8e4
AF = mybir.ActivationFunctionType
PM = mybir.MatmulPerfMode
ALU = mybir.AluOpType

# psum scores arrive scaled by LAM = lam_q*lam_k (power-folded into the fp8
# hi/lo weights to stay in e4m3's normal range); exp undoes it via
# activation-scale / pow-base.
LAM = 2048.0
GAMMA = 1.0 / LAM

_nc_cache = None

# engine split tables (tuned against the cost-model trace)
# exp engine per (br, u*8+m); mul engine per (br, u*8+m)
# walrus: GPSIMD cannot access PSUM -> Pool only gets SBUF-side est*em muls;
# exps (psum reads) split Act/DVE, copies on Act, otu/rcp on DVE.
# walrus codegen rejects ALU pow on DVE too -> all exps via Act activation.
EXP0 = ["A"] * 8
EXP1 = ["A"] * 8
MUL0 = ["P", "P", "P", "V", "P", "P", "P", "V"]
MUL1 = ["P", "P", "P", "V", "P", "P", "P", "V"]


def _build(reps=1):
    nc = bacc.Bacc("TRN2", target_bir_lowering=False, debug=False, num_devices=8)
    x8 = nc.declare_dram_parameter("x8", [2 * 4 * 256, N], FP8, isOutput=False)
    w8 = nc.declare_dram_parameter("w8", [2 * 4 * 256, 3 * NHALF], FP8, isOutput=False)
    em0 = nc.declare_dram_parameter("em0", [N, N], BF16, isOutput=False)
    em1 = nc.declare_dram_parameter("em1", [N, N], BF16, isOutput=False)
    pw8 = nc.declare_dram_parameter("pw8", [2 * 2 * 256, C], FP8, isOutput=False)
    out0 = nc.declare_dram_parameter("out0", [C, N], BF16, isOutput=True)
    out1 = nc.declare_dram_parameter("out1", [C, N], BF16, isOutput=True)

    with tile.TileContext(nc) as tc:
        for _ in range(reps):
            with ExitStack() as ctx:
                _body(tc, ctx, x8, w8, em0, em1, pw8, out0, out1)
    nc.compile()
    return nc


def _body(tc, ctx, x8, w8, em0, em1, pw8, out0, out1):
    nc = tc.nc

    pers = ctx.enter_context(tc.tile_pool(name="pers", bufs=1))
    work = ctx.enter_context(tc.tile_pool(name="work", bufs=1))
    psum = ctx.enter_context(tc.tile_pool(name="psum", bufs=1, space="PSUM"))

    EW = {"A": nc.scalar, "V": nc.vector, "P": nc.gpsimd}

    def ew_copy(e, dst, src):
        if e == "A":
            nc.scalar.copy(dst, src)
        else:
            EW[e].tensor_copy(dst, src)

    # ---------------- persistent tiles ----------------
    # x8t[hl][g], w8t[hl][g]: fp8 hi/lo chunk-pair tiles for DoubleRow
    x8t = [[pers.tile([P, 2, N], FP8, name=f"x{hl}{g}", tag=f"x{hl}{g}")
            for g in range(4)] for hl in range(2)]
    w8t = [[pers.tile([P, 2, 3 * NHALF], FP8, name=f"w{hl}{g}", tag=f"w{hl}{g}")
            for g in range(4)] for hl in range(2)]
    qk_t = [pers.tile([P, N], BF16, name=f"qk{i}", tag=f"qk{i}") for i in range(8)]
    vd = pers.tile([P, 8, NHALF], BF16, name="vd", tag="vd")
    ones = pers.tile([P, 1], BF16, name="ones", tag="ones")
    vT = [pers.tile([P, N], BF16, name=f"vT{g}", tag=f"vT{g}") for g in range(4)]
    em_t = [[pers.tile([P, N], BF16, name=f"em{br}_{m}", tag=f"em{br}_{m}")
             for m in range(8)] for br in range(2)]
    Bt = pers.tile([P, N], F32, name="Bt", tag="Bt")
    ot_t = [pers.tile([P, N], BF16, name=f"ot{i}", tag=f"ot{i}") for i in range(8)]
    # hi/lo fp8 pair-tiles for the proj DR matmuls (t = ci chunk-pair; br)
    ot8 = [[[pers.tile([P, 2, N], FP8, name=f"ot8{hl}{br}{t}", tag=f"ot8{hl}{br}{t}")
             for t in range(2)] for br in range(2)] for hl in range(2)]
    pw_t = [[pers.tile([P, 2, C], FP8, name=f"pw{hl}{t}", tag=f"pw{hl}{t}")
             for t in range(2)] for hl in range(2)]

    def pts_tile(ui):
        return work.tile([P, 8, N], BF16, name=f"pts{ui}", tag="pts", bufs=3)

    def st_ps(nm):
        return psum.tile([P, N], F32, name=nm, tag="st", bufs=2)

    def pv_ps(nm):
        return psum.tile([P, 2, NHALF], F32, name=nm, tag="pv", bufs=2)

    # ---------------- input DMA staging ----------------
    # g0 first so A1 pair-0 can start immediately; issue on SP + helpers.
    def dma_x(hl, g, eng=nc.sync):
        src = x8[(hl * 4 + g) * 256:(hl * 4 + g + 1) * 256, :]
        eng.dma_start(x8t[hl][g][:], src.rearrange("(p two) n -> p two n", two=2))

    def dma_w(hl, g, eng=nc.sync):
        src = w8[(hl * 4 + g) * 256:(hl * 4 + g + 1) * 256, :]
        eng.dma_start(w8t[hl][g][:], src.rearrange("(p two) n -> p two n", two=2))

    def dma_em(br, m, eng=nc.sync):
        src = em0 if br == 0 else em1
        eng.dma_start(em_t[br][m][:], src[m * P:(m + 1) * P, :])

    for g in range(2):
        dma_w(0, g, nc.sync)
        dma_x(0, g, nc.gpsimd)
        dma_w(1, g, nc.gpsimd)
        dma_x(1, g, nc.sync)
    nc.vector.memset(Bt[:], float(np.exp(GAMMA)))
    nc.vector.memset(ones[:], 1.0)
    for g in range(2, 4):
        dma_w(0, g, nc.sync)
        dma_x(0, g, nc.gpsimd)
        dma_w(1, g, nc.gpsimd)
        dma_x(1, g, nc.sync)
    for m in range(8):
        dma_em(0, m, nc.sync if m % 2 == 0 else nc.gpsimd)
        dma_em(1, m, nc.sync if m % 2 == 1 else nc.gpsimd)

    # ---------------- fill closures ----------------
    def hilo_mms(ps_half, lhs_of, rhs_of, first, last):
        """3-cross-term hi/lo fp8 DR accumulation over 4 chunk-pairs."""
        for g in range(4):
            combos = ((0, 0), (1, 0), (0, 1))
            for ci, (lh, rh) in enumerate(combos):
                nc.tensor.matmul(
                    ps_half,
                    lhsT=lhs_of(lh, g), rhs=rhs_of(rh, g),
                    start=(first and g == 0 and ci == 0),
                    stop=(last and g == 3 and ci == 2),
                    perf_mode=PM.DoubleRow,
                )

    def a1_fill(cc, nh, ceng):
        ps = st_ps(f"a1_{cc}_{nh}")
        half = ps[:, nh * NHALF:(nh + 1) * NHALF]
        hilo_mms(
            half,
            lambda hl, g: w8t[hl][g][:, :, cc * P:(cc + 1) * P],
            lambda hl, g: x8t[hl][g][:, :, nh * NHALF:(nh + 1) * NHALF],
            True, True,
        )
        ew_copy(ceng, qk_t[cc][:, nh * NHALF:(nh + 1) * NHALF], half)

    def a2_fill(m, ceng):
        ps = st_ps(f"a2_{m}")
        half = ps[:, 0:NHALF]
        hilo_mms(
            half,
            lambda hl, g: x8t[hl][g][:, :, m * P:(m + 1) * P],
            lambda hl, g: w8t[hl][g][:, :, 2 * NHALF:3 * NHALF],
            True, True,
        )
        ew_copy(ceng, vd[:, m, :], half)
        for g in range(4):
            nc.sync.dma_start(
                vT[g][:, m * P:(m + 1) * P],
                vd[:, m, g * P:(g + 1) * P],
                transpose=True,
            )

    def proj_fill(br, mt):
        ps = st_ps(f"y{br}_{mt}")
        out = out0 if br == 0 else out1
        for nh in range(2):
            sl = slice(nh * NHALF, (nh + 1) * NHALF)
            for t in range(2):
                for ci, (lh, rh) in enumerate(((0, 0), (1, 0), (0, 1))):
                    nc.tensor.matmul(
                        ps[:, sl],
                        lhsT=pw_t[lh][t][:, :, mt * P:(mt + 1) * P],
                        rhs=ot8[rh][br][t][:, :, sl],
                        start=(t == 0 and ci == 0), stop=(t == 1 and ci == 2),
                        perf_mode=PM.DoubleRow,
                    )
        y = work.tile([P, N], BF16, name="y", tag="y", bufs=2)
        for nh in range(2):
            sl = slice(nh * NHALF, (nh + 1) * NHALF)
            ew_copy("V", y[:, sl], ps[:, sl])
            nc.sync.dma_start(out[mt * P:(mt + 1) * P, sl], y[:, sl])

    # Deadline-paced fillers (gstep -> closure)
    sched = {}
    cp_cyc = ["V", "V", "V", "V", "V", "V", "V", "V"]
    for m in range(8):
        sched[m] = (lambda m=m, e=cp_cyc[m]: a2_fill(m, e))
    gs = {1: [9, 13, 17, 21], 2: [25, 29, 33, 37], 3: [41, 45, 49, 53]}
    for pair in (1, 2, 3):
        fills = [(cc, nh) for nh in range(2) for cc in (pair, 4 + pair)]
        for g, (cc, nh) in zip(gs[pair], fills):
            e = cp_cyc[(cc + nh) % 8]
            sched[g] = (lambda cc=cc, nh=nh, e=e: a1_fill(cc, nh, e))

    def late_dmas():
        for hl in range(2):
            for t in range(2):
                src_ = pw8[(hl * 2 + t) * 256:(hl * 2 + t + 1) * 256, :]
                nc.sync.dma_start(pw_t[hl][t][:],
                                  src_.rearrange("(p two) n -> p two n", two=2))
    sched[55] = late_dmas

    # upfront A1 for head-pair 0
    for nh in range(2):
        for cc in (0, 4):
            a1_fill(cc, nh, cp_cyc[(cc + nh) % 8])

    # ---------------- attention units ----------------
    UNITS = [(0, 0), (1, 0), (0, 1), (1, 1), (0, 2), (1, 2),
             (0, 3), (1, 3), (0, 4), (1, 4), (0, 5), (0, 6),
             (0, 7), (1, 5), (1, 6), (1, 7)]
    pts = {}

    def emit_pv(u_state, m):
        u, br, h, pv = u_state
        est = pts[u % 3]
        pv4 = pv[:, :, 0:260].rearrange("p b (j c) -> p b j c", c=65)
        for j in range(8):
            lhsT = est[:, m, j * P:(j + 1) * P]
            nc.tensor.matmul(
                pv4[:, j // 4, j % 4, 0:64],
                lhsT=lhsT, rhs=vd[:, m, 64 * h:64 * h + 64],
                start=(m == 0 and j % 4 == 0), stop=False,
            )
            nc.tensor.matmul(
                pv4[:, j // 4, j % 4, 64:65],
                lhsT=lhsT, rhs=ones[:],
                start=False, stop=(m == 7 and j % 4 == 3),
            )

    otu_pair = {}

    def finish_unit(u_state):
        u, br, h, pv = u_state
        g = br * 4 + h // 2
        pv4 = pv[:, :, 0:260].rearrange("p b (j c) -> p b j c", c=65)
        rcp = work.tile([P, 2, 4, 1], F32, name="rcp", tag="rcp", bufs=2)
        nc.vector.reciprocal(rcp[:], pv4[:, :, :, 64:65])
        if (br, h // 2) not in otu_pair:
            otu_pair[(br, h // 2)] = work.tile(
                [P, 2, 4, 2, 64], BF16, name=f"otu{u}", tag="otu", bufs=3)
        otu = otu_pair[(br, h // 2)]
        nc.vector.tensor_mul(
            otu[:, :, :, h % 2, :], pv4[:, :, :, 0:64],
            rcp[:].broadcast_to((P, 2, 4, 64)))
        if h % 2 == 1:
            of = otu[:].rearrange("p b j two c -> p (b j) (two c)")
            for j in range(8):
                nc.sync.dma_start(
                    ot_t[g][:, j * P:(j + 1) * P],
                    of[:, j, :],
                    transpose=True,
                )
            t, i = (h // 2) // 2, (h // 2) % 2
            ce = "P" if (h // 2) % 2 == 0 else "V"
            EW[ce].tensor_copy(ot8[0][br][t][:, i, :], ot_t[g][:])
            EW[ce].tensor_sub(ot8[1][br][t][:, i, :], ot_t[g][:],
                              ot8[0][br][t][:, i, :])

    proj_q = []
    gstep = 0
    hist = []
    for ui, (br, h) in enumerate(UNITS):
        if br == 0:
            kT, qT, ro = qk_t[4 + h // 2], qk_t[h // 2], (h % 2) * 64
        else:
            kT = qT = vT[h // 2]
            ro = (h % 2) * 64
        pv = pv_ps(f"pv{ui}")
        cur = (ui, br, h, pv)
        est = pts_tile(ui)
        pts[ui % 3] = est
        for m in range(8):
            if br == 0:
                ps = st_ps(f"st{ui}_{m}")
                for nh in range(2):
                    nc.tensor.matmul(
                        ps[:, nh * NHALF:(nh + 1) * NHALF],
                        lhsT=kT[ro:ro + 64, m * P:(m + 1) * P],
                        rhs=qT[ro:ro + 64, nh * NHALF:(nh + 1) * NHALF],
                        start=True, stop=True,
                    )
                e = EXP0[m]
                if e == "A":
                    nc.scalar.activation(est[:, m, :], ps[:], AF.Exp, scale=GAMMA)
                else:
                    EW[e].tensor_tensor(est[:, m, :], Bt[:], ps[:], ALU.pow)
                me = MUL0[m]
                EW[me].tensor_mul(est[:, m, :], est[:, m, :], em_t[0][m][:])
            else:
                mi = m
                ps = st_ps(f"st{ui}_{m}")
                if mi < 4:
                    nc.tensor.matmul(
                        ps[:, mi * P:NHALF],
                        lhsT=kT[ro:ro + 64, mi * P:(mi + 1) * P],
                        rhs=qT[ro:ro + 64, mi * P:NHALF],
                        start=True, stop=True,
                    )
                    nc.tensor.matmul(
                        ps[:, NHALF:N],
                        lhsT=kT[ro:ro + 64, mi * P:(mi + 1) * P],
                        rhs=qT[ro:ro + 64, NHALF:N],
                        start=True, stop=True,
                    )
                else:
                    nc.tensor.matmul(
                        ps[:, mi * P:N],
                        lhsT=kT[ro:ro + 64, mi * P:(mi + 1) * P],
                        rhs=qT[ro:ro + 64, mi * P:N],
                        start=True, stop=True,
                    )
                e = EXP1[m]
                if e == "A":
                    nc.scalar.activation(est[:, mi, mi * P:N], ps[:, mi * P:N],
                                         AF.Exp, scale=GAMMA)
                else:
                    EW[e].tensor_tensor(est[:, mi, mi * P:N], Bt[:, mi * P:N],
                                        ps[:, mi * P:N], ALU.pow)
                for k in range(mi):
                    nc.sync.dma_start(
                        est[:, mi, k * P:(k + 1) * P],
                        est[:, k, mi * P:(mi + 1) * P],
                        transpose=True,
                    )
            if len(hist) >= 2:
                emit_pv(hist[-2], m)
            if gstep in sched:
                sched.pop(gstep)()
            if proj_q and gstep % 3 == 0:
                proj_q.pop()()
            gstep += 1
        if br == 1:
            # est*em only after all mirrors read the pure-exp tiles
            for m in range(8):
                EW[MUL1[m]].tensor_mul(est[:, m, :], est[:, m, :], em_t[1][m][:])
        hist.append(cur)
        if len(hist) >= 3:
            finish_unit(hist[-3])
            if (hist[-3][1], hist[-3][2]) == (0, 7):
                proj_q = [(lambda mt=mt: proj_fill(0, mt)) for mt in range(8)]
                proj_q.reverse()

    for u_state in hist[-2:]:
        for m in range(8):
            emit_pv(u_state, m)
        finish_unit(u_state)
        while proj_q:
            proj_q.pop()()
    for g in sorted(sched):
        sched.pop(g)()

    # ---------------- branch-1 projection (tail) ----------------
    for mt in range(8):
        proj_fill(1, mt)


def _hilo(a):
    """Split f32 array into fp8_e4m3 hi + lo residual."""
    hi = a.astype(ml_dtypes.float8_e4m3)
    lo = (a - hi.astype(np.float32)).astype(ml_dtypes.float8_e4m3)
    return hi, lo


def _pack_pairs(a):
    """[1024, cols] -> [4, 128, 2, cols] chunk-pair layout, flattened to
    [2048?, cols] rows ((g*128+p)*2+i)."""
    ch = a.reshape(4, 2, 128, -1).transpose(0, 2, 1, 3)  # g, p, i, cols
    return np.ascontiguousarray(ch.reshape(4 * 128 * 2, -1))


def _prep_inputs(x, attn_mask, qkv_w, proj_w, proj_b):
    """8 per-core input maps: core (b, hh) = batch b, head-half hh."""
    bf = ml_dtypes.bfloat16
    q_w, k_w, v_w = qkv_w[0:C], qkv_w[C:2 * C], qkv_w[2 * C:3 * C]
    s = float(D ** (-0.5))
    em0 = np.ascontiguousarray(np.exp(attn_mask[0, 0]).T.astype(bf))
    em1 = np.ascontiguousarray(np.exp(attn_mask[1, 0]).T.astype(bf))
    # scale folding: q' = 64*s*q, k' = 32*k, v' = sqrt(2048*s)*v so that
    # q'k' = 2048*s*qk and v'v' = 2048*s*vv (GAMMA=1/2048 undone in exp),
    # while keeping the fp8 hi/lo weights in e4m3's normal range.
    # lam_v*sqrt(s) = 16 exactly, compensated in pwT.
    lam_q, lam_k = 64.0, 32.0
    lam_v = float(np.sqrt(2048.0 * s))
    in_maps = []
    xp = {}
    for core in range(8):
        b, hh = core // 2, core % 2
        sl = slice(hh * NHALF, (hh + 1) * NHALF)
        if b not in xp:
            xT = np.ascontiguousarray(x[b].T.astype(np.float32))
            xh, xl = _hilo(xT)
            xp[b] = np.vstack([_pack_pairs(xh), _pack_pairs(xl)])
        wcat = np.hstack([(q_w[sl] * (s * lam_q)).T, (k_w[sl] * lam_k).T,
                          (v_w[sl] * lam_v).T]).astype(np.float32)
        wh, wl = _hilo(wcat)
        w8 = np.vstack([_pack_pairs(wh), _pack_pairs(wl)])
        # lam_p=256 keeps pw in e4m3's normal range; host divides it out
        pwTf = (proj_w[:, sl] * (256.0 / 16.0)).T.astype(np.float32)
        ph, pl = _hilo(pwTf)
        pw8c = np.vstack([
            np.ascontiguousarray(h_.reshape(2, 2, 128, C).transpose(0, 2, 1, 3)
                                 .reshape(2 * 256, C))
            for h_ in (ph, pl)])
        in_maps.append({
            "x8": xp[b], "w8": w8, "em0": em0, "em1": em1, "pw8": pw8c,
        })
    return in_maps


def _run(inputs, trace=False, **kw):
    global _nc_cache
    from concourse.bass_utils import run_bass_kernel_spmd
    if _nc_cache is None:
        _nc_cache = _build()
    in_maps = _prep_inputs(**inputs)
    res = run_bass_kernel_spmd(_nc_cache, in_maps, core_ids=list(range(8)),
                               trace=trace, **kw)
    pb = np.asarray(inputs["proj_b"], dtype=np.float32)
    outs = []
    for br in range(2):
        nm = f"out{br}"
        ys = []
        for b in range(B):
            p0 = np.asarray(res.results[2 * b][nm], dtype=np.float32)
            p1 = np.asarray(res.results[2 * b + 1][nm], dtype=np.float32)
            ys.append((p0 + p1).T / 256.0 + pb)
        outs.append(np.stack(ys))
    x_ori, x_v = outs[0], outs[1]
    return (x_v, x_ori), res


def kernel(x, attn_mask, qkv_w, proj_w, proj_b):
    (x_v, x_ori), _ = _run(dict(x=np.asarray(x), attn_mask=np.asarray(attn_mask),
                                qkv_w=np.asarray(qkv_w), proj_w=np.asarray(proj_w),
                                proj_b=np.asarray(proj_b)))
    return (x_v, x_ori)


# revision 38
# speedup vs baseline: 1.1440x; 1.1041x over previous
"""Head-parallel dual-branch attention kernel for one TRN2 chip (8 cores).

Sharding: core (b, hh) = batch b (0-3) x head-half hh (0-1).  Each core
computes BOTH branches for its 8 heads and emits per-branch yT-partials
(proj row-sharded over channels, f32); the host sums the two head-half
partials and adds the bias during unshard.  Zero device comm.

Key structure (per core, SPMD-uniform):
  A1: qk [1024, N] = [q|k]-half weights^T @ x^T, via fp8 hi+lo residual
      pairs and DoubleRow matmuls (3 cross terms per 256-chunk = 0.75x
      the bf16 PE cost at ~bf16 accuracy).
  A2: vn [N, 8*65] = x @ v-half^T (+ ones col per head) same hi/lo trick;
      vT per head-pair via SBUF->SBUF DMA transpose of vn (bf16).
  per unit (br, h) of 16 (br0 units first, then br1):
    S[m-block, n] = lhsT(kT|vT) x rhs(qT|vT), K=64, bf16   (br1: upper
      blocks only; symmetric part mirrored in est domain via DMA
      transpose - est is symmetric for (v,v) before masking)
    est = exp(S) on Act (activation, scale=1) or DVE/Pool
      (tensor_tensor pow: e_tile ** S) - exp work is split across all
      three elementwise engines.
    est *= em[br] (exp of mask, DVE/Pool)
    PV[n, 8x65] += est(m)^T x vn(m) (65th col = ones -> denominator)
    OT[n, 512] = PV[:, :64] * recip(PV[:, 64])  per head
    ot[c, n] via DMA transpose of OT (no PE transpose)
  proj: yT_br[co, n] += pwT^T @ ot, bf16; psum -> DRAM f32 directly.

Host adds proj bias and sums head-half partials in f32.
"""

import numpy as np
import ml_dtypes

import concourse.bass as bass
from concourse import bacc
import concourse.tile as tile
import concourse.mybir as mybir
from contextlib import ExitStack

B, N, C, H, D, P = 4, 1024, 1024, 16, 64, 128
HH = 8          # heads per core
NHALF = 512
BF16 = mybir.dt.bfloat16
F32 = mybir.dt.float32
FP8 = mybir.dt.float8e4
AF = mybir.ActivationFunctionType
PM = mybir.MatmulPerfMode
ALU = mybir.AluOpType

# psum scores arrive scaled by LAM = lam_q*lam_k (power-folded into the fp8
# hi/lo weights to stay in e4m3's normal range); exp undoes it via
# activation-scale / pow-base.
LAM = 2048.0
GAMMA = 1.0 / LAM

_nc_cache = None

# engine split tables (tuned against the cost-model trace)
# exp engine per (br, u*8+m); mul engine per (br, u*8+m)
# walrus: GPSIMD cannot access PSUM -> Pool only gets SBUF-side est*em muls;
# exps (psum reads) split Act/DVE, copies on Act, otu/rcp on DVE.
# walrus codegen rejects ALU pow on DVE too -> all exps via Act activation.
EXP0 = ["A"] * 8
EXP1 = ["A"] * 8
MUL0 = ["P", "P", "P", "V", "P", "P", "P", "V"]
MUL1 = ["P", "P", "P", "V", "P", "P", "P", "V"]


def _build(reps=1):
    nc = bacc.Bacc("TRN2", target_bir_lowering=False, debug=False, num_devices=8)
    x8 = nc.declare_dram_parameter("x8", [2 * 4 * 256, N], FP8, isOutput=False)
    w8 = nc.declare_dram_parameter("w8", [2 * 4 * 256, 3 * NHALF], FP8, isOutput=False)
    em0 = nc.declare_dram_parameter("em0", [N, N], BF16, isOutput=False)
    em1 = nc.declare_dram_parameter("em1", [N, N], BF16, isOutput=False)
    pw8 = nc.declare_dram_parameter("pw8", [2 * 2 * 256, C], FP8, isOutput=False)
    out0 = nc.declare_dram_parameter("out0", [C, N], BF16, isOutput=True)
    out1 = nc.declare_dram_parameter("out1", [C, N], BF16, isOutput=True)

    with tile.TileContext(nc) as tc:
        for _ in range(reps):
            with ExitStack() as ctx:
                _body(tc, ctx, x8, w8, em0, em1, pw8, out0, out1)
    nc.compile()
    return nc


def _body(tc, ctx, x8, w8, em0, em1, pw8, out0, out1):
    nc = tc.nc

    pers = ctx.enter_context(tc.tile_pool(name="pers", bufs=1))
    work = ctx.enter_context(tc.tile_pool(name="work", bufs=1))
    psum = ctx.enter_context(tc.tile_pool(name="psum", bufs=1, space="PSUM"))

    EW = {"A": nc.scalar, "V": nc.vector, "P": nc.gpsimd}

    def ew_copy(e, dst, src):
        if e == "A":
            nc.scalar.copy(dst, src)
        else:
            EW[e].tensor_copy(dst, src)

    # ---------------- persistent tiles ----------------
    # x8t[hl][g], w8t[hl][g]: fp8 hi/lo chunk-pair tiles for DoubleRow
    x8t = [[pers.tile([P, 2, N], FP8, name=f"x{hl}{g}", tag=f"x{hl}{g}")
            for g in range(4)] for hl in range(2)]
    w8t = [[pers.tile([P, 2, 3 * NHALF], FP8, name=f"w{hl}{g}", tag=f"w{hl}{g}")
            for g in range(4)] for hl in range(2)]
    qk_t = [pers.tile([P, N], BF16, name=f"qk{i}", tag=f"qk{i}") for i in range(8)]
    vd = pers.tile([P, 8, NHALF], BF16, name="vd", tag="vd")
    ones = pers.tile([P, 1], BF16, name="ones", tag="ones")
    vT = [pers.tile([P, N], BF16, name=f"vT{g}", tag=f"vT{g}") for g in range(4)]
    em_t = [[pers.tile([P, N], BF16, name=f"em{br}_{m}", tag=f"em{br}_{m}")
             for m in range(8)] for br in range(2)]
    Bt = pers.tile([P, N], F32, name="Bt", tag="Bt")
    ot_t = [pers.tile([P, N], BF16, name=f"ot{i}", tag=f"ot{i}") for i in range(8)]
    # hi/lo fp8 pair-tiles for the proj DR matmuls (t = ci chunk-pair; br)
    ot8 = [[[pers.tile([P, 2, N], FP8, name=f"ot8{hl}{br}{t}", tag=f"ot8{hl}{br}{t}")
             for t in range(2)] for br in range(2)] for hl in range(2)]
    pw_t = [[pers.tile([P, 2, C], FP8, name=f"pw{hl}{t}", tag=f"pw{hl}{t}")
             for t in range(2)] for hl in range(2)]

    def pts_tile(ui):
        return work.tile([P, 8, N], BF16, name=f"pts{ui}", tag="pts", bufs=3)

    def st_ps(nm):
        return psum.tile([P, N], F32, name=nm, tag="st", bufs=3)

    def pv_ps(nm):
        return psum.tile([P, 2, NHALF], F32, name=nm, tag="pv", bufs=1)

    # ---------------- input DMA staging ----------------
    # g0 first so A1 pair-0 can start immediately; issue on SP + helpers.
    def dma_x(hl, g, eng=nc.sync, half=None):
        src = x8[(hl * 4 + g) * 256:(hl * 4 + g + 1) * 256, :]
        sl = slice(0, NHALF) if half == 0 else (
            slice(NHALF, N) if half == 1 else slice(0, N))
        eng.dma_start(x8t[hl][g][:, :, sl],
                      src[:, sl].rearrange("(p two) n -> p two n", two=2))

    def dma_w(hl, g, eng=nc.sync, half=None):
        src = w8[(hl * 4 + g) * 256:(hl * 4 + g + 1) * 256, :]
        sl = slice(0, 768) if half == 0 else (
            slice(768, 3 * NHALF) if half == 1 else slice(0, 3 * NHALF))
        eng.dma_start(w8t[hl][g][:, :, sl],
                      src[:, sl].rearrange("(p two) n -> p two n", two=2))

    def dma_em(br, m, eng=nc.sync):
        src = em0 if br == 0 else em1
        eng.dma_start(em_t[br][m][:], src[m * P:(m + 1) * P, :])

    for g in range(4):
        dma_w(0, g, nc.sync, half=0)
        dma_x(0, g, nc.gpsimd, half=0)
        dma_w(1, g, nc.gpsimd, half=0)
        dma_x(1, g, nc.sync, half=0)
    nc.vector.memset(Bt[:], float(np.exp(GAMMA)))
    nc.vector.memset(ones[:], 1.0)
    # warm the Act exp table during startup (LoadActFuncSet is 1.3us)
    actw = pers.tile([P, 1], BF16, name="actw", tag="actw")
    nc.scalar.activation(actw[:], ones[:], AF.Exp)
    for g in range(4):
        dma_w(0, g, nc.sync, half=1)
        dma_x(0, g, nc.gpsimd, half=1)
        dma_w(1, g, nc.gpsimd, half=1)
        dma_x(1, g, nc.sync, half=1)
    for m in range(8):
        dma_em(0, m, nc.sync if m % 2 == 0 else nc.gpsimd)
        dma_em(1, m, nc.sync if m % 2 == 1 else nc.gpsimd)

    # ---------------- fill closures ----------------
    def hilo_mms(ps_half, lhs_of, rhs_of, first, last):
        """3-cross-term hi/lo fp8 DR accumulation over 4 chunk-pairs."""
        for g in range(4):
            combos = ((0, 0), (1, 0), (0, 1))
            for ci, (lh, rh) in enumerate(combos):
                nc.tensor.matmul(
                    ps_half,
                    lhsT=lhs_of(lh, g), rhs=rhs_of(rh, g),
                    start=(first and g == 0 and ci == 0),
                    stop=(last and g == 3 and ci == 2),
                    perf_mode=PM.DoubleRow,
                )

    def a1_fill(cc, nh, ceng):
        ps = st_ps(f"a1_{cc}_{nh}")
        half = ps[:, nh * NHALF:(nh + 1) * NHALF]
        hilo_mms(
            half,
            lambda hl, g: w8t[hl][g][:, :, cc * P:(cc + 1) * P],
            lambda hl, g: x8t[hl][g][:, :, nh * NHALF:(nh + 1) * NHALF],
            True, True,
        )
        ew_copy(ceng, qk_t[cc][:, nh * NHALF:(nh + 1) * NHALF], half)

    def a2_fill(m, ceng):
        ps = st_ps(f"a2_{m}")
        half = ps[:, 0:NHALF]
        hilo_mms(
            half,
            lambda hl, g: x8t[hl][g][:, :, m * P:(m + 1) * P],
            lambda hl, g: w8t[hl][g][:, :, 2 * NHALF:3 * NHALF],
            True, True,
        )
        ew_copy(ceng, vd[:, m, :], half)
        for g in range(4):
            nc.sync.dma_start(
                vT[g][:, m * P:(m + 1) * P],
                vd[:, m, g * P:(g + 1) * P],
                transpose=True,
            )

    def proj_fill(br, mt):
        ps = st_ps(f"y{br}_{mt}")
        out = out0 if br == 0 else out1
        for nh in range(2):
            sl = slice(nh * NHALF, (nh + 1) * NHALF)
            for t in range(2):
                for ci, (lh, rh) in enumerate(((0, 0), (1, 0), (0, 1))):
                    nc.tensor.matmul(
                        ps[:, sl],
                        lhsT=pw_t[lh][t][:, :, mt * P:(mt + 1) * P],
                        rhs=ot8[rh][br][t][:, :, sl],
                        start=(t == 0 and ci == 0), stop=(t == 1 and ci == 2),
                        perf_mode=PM.DoubleRow,
                    )
        y = work.tile([P, N], BF16, name="y", tag="y", bufs=2)
        for nh in range(2):
            sl = slice(nh * NHALF, (nh + 1) * NHALF)
            ew_copy("V" if br == 0 else "A", y[:, sl], ps[:, sl])
            nc.sync.dma_start(out[mt * P:(mt + 1) * P, sl], y[:, sl])

    # Deadline-paced fillers (gstep -> closure)
    sched = {}
    cp_cyc = ["V", "V", "V", "V", "V", "V", "V", "V"]
    for m in range(8):
        sched[m] = (lambda m=m: a2_fill(m, "A"))
    gs = {1: [9, 13, 17, 21], 2: [50, 54, 58, 62], 3: [74, 78, 82, 86]}
    for pair in (1, 2, 3):
        fills = [(cc, nh) for nh in range(2) for cc in (pair, 4 + pair)]
        for g, (cc, nh) in zip(gs[pair], fills):
            e = cp_cyc[(cc + nh) % 8]
            sched[g] = (lambda cc=cc, nh=nh, e=e: a1_fill(cc, nh, e))

    def late_dmas():
        for hl in range(2):
            for t in range(2):
                src_ = pw8[(hl * 2 + t) * 256:(hl * 2 + t + 1) * 256, :]
                nc.sync.dma_start(pw_t[hl][t][:],
                                  src_.rearrange("(p two) n -> p two n", two=2))
    sched[67] = late_dmas

    # upfront A1 for head-pair 0
    for nh in range(2):
        for cc in (0, 4):
            a1_fill(cc, nh, "A")

    # ---------------- attention units ----------------
    UNITS = [(0, 0), (1, 0), (0, 1), (1, 1), (0, 2), (1, 2),
             (0, 3), (1, 3), (0, 4), (0, 5), (1, 4), (0, 6),
             (0, 7), (1, 5), (1, 6), (1, 7)]
    pts = {}

    def emit_pv(u_state, m):
        u, br, h, pv = u_state
        est = pts[u % 3]
        pv4 = pv[:, :, 0:260].rearrange("p b (j c) -> p b j c", c=65)
        for j in range(8):
            lhsT = est[:, m, j * P:(j + 1) * P]
            nc.tensor.matmul(
                pv4[:, j // 4, j % 4, 0:64],
                lhsT=lhsT, rhs=vd[:, m, 64 * h:64 * h + 64],
                start=(m == 0 and j % 4 == 0), stop=False,
            )
            nc.tensor.matmul(
                pv4[:, j // 4, j % 4, 64:65],
                lhsT=lhsT, rhs=ones[:],
                start=False, stop=(m == 7 and j % 4 == 3),
            )

    otu_pair = {}

    def finish_unit(u_state):
        u, br, h, pv = u_state
        g = br * 4 + h // 2
        pv4 = pv[:, :, 0:260].rearrange("p b (j c) -> p b j c", c=65)
        rcp = work.tile([P, 2, 4, 1], F32, name="rcp", tag="rcp", bufs=2)
        nc.vector.reciprocal(rcp[:], pv4[:, :, :, 64:65])
        if (br, h // 2) not in otu_pair:
            otu_pair[(br, h // 2)] = work.tile(
                [P, 2, 4, 2, 64], BF16, name=f"otu{u}", tag="otu", bufs=3)
        otu = otu_pair[(br, h // 2)]
        nc.vector.tensor_mul(
            otu[:, :, :, h % 2, :], pv4[:, :, :, 0:64],
            rcp[:].broadcast_to((P, 2, 4, 64)))
        if h % 2 == 1:
            of = otu[:].rearrange("p b j two c -> p (b j) (two c)")
            for j in range(8):
                nc.sync.dma_start(
                    ot_t[g][:, j * P:(j + 1) * P],
                    of[:, j, :],
                    transpose=True,
                )
            t, i = (h // 2) // 2, (h // 2) % 2
            ce = "P" if (h // 2) % 2 == 0 else "V"
            EW[ce].tensor_copy(ot8[0][br][t][:, i, :], ot_t[g][:])
            EW[ce].tensor_sub(ot8[1][br][t][:, i, :], ot_t[g][:],
                              ot8[0][br][t][:, i, :])

    proj_q = []
    gstep = 0
    hist = []
    for ui, (br, h) in enumerate(UNITS):
        if br == 0:
            kT, qT, ro = qk_t[4 + h // 2], qk_t[h // 2], (h % 2) * 64
        else:
            kT = qT = vT[h // 2]
            ro = (h % 2) * 64
        pv = pv_ps(f"pv{ui}")
        cur = (ui, br, h, pv)
        est = pts_tile(ui)
        pts[ui % 3] = est
        for m in range(8):
            if br == 0:
                ps = st_ps(f"st{ui}_{m}")
                for nh in range(2):
                    nc.tensor.matmul(
                        ps[:, nh * NHALF:(nh + 1) * NHALF],
                        lhsT=kT[ro:ro + 64, m * P:(m + 1) * P],
                        rhs=qT[ro:ro + 64, nh * NHALF:(nh + 1) * NHALF],
                        start=True, stop=True,
                    )
                e = EXP0[m]
                if e == "A":
                    nc.scalar.activation(est[:, m, :], ps[:], AF.Exp, scale=GAMMA)
                else:
                    EW[e].tensor_tensor(est[:, m, :], Bt[:], ps[:], ALU.pow)
                me = MUL0[m]
                EW[me].tensor_mul(est[:, m, :], est[:, m, :], em_t[0][m][:])
            else:
                mi = m
                ps = st_ps(f"st{ui}_{m}")
                if mi < 4:
                    nc.tensor.matmul(
                        ps[:, mi * P:NHALF],
                        lhsT=kT[ro:ro + 64, mi * P:(mi + 1) * P],
                        rhs=qT[ro:ro + 64, mi * P:NHALF],
                        start=True, stop=True,
                    )
                    nc.tensor.matmul(
                        ps[:, NHALF:N],
                        lhsT=kT[ro:ro + 64, mi * P:(mi + 1) * P],
                        rhs=qT[ro:ro + 64, NHALF:N],
                        start=True, stop=True,
                    )
                else:
                    nc.tensor.matmul(
                        ps[:, mi * P:N],
                        lhsT=kT[ro:ro + 64, mi * P:(mi + 1) * P],
                        rhs=qT[ro:ro + 64, mi * P:N],
                        start=True, stop=True,
                    )
                e = EXP1[m]
                if e == "A":
                    nc.scalar.activation(est[:, mi, mi * P:N], ps[:, mi * P:N],
                                         AF.Exp, scale=GAMMA)
                else:
                    EW[e].tensor_tensor(est[:, mi, mi * P:N], Bt[:, mi * P:N],
                                        ps[:, mi * P:N], ALU.pow)
                for k in range(mi):
                    nc.sync.dma_start(
                        est[:, mi, k * P:(k + 1) * P],
                        est[:, k, mi * P:(mi + 1) * P],
                        transpose=True,
                    )
            if len(hist) >= 2:
                emit_pv(hist[-2], m)
            if gstep in sched:
                sched.pop(gstep)()
            if proj_q and gstep % 2 == 0:
                proj_q.pop()()
            gstep += 1
        if br == 1:
            # est*em only after all mirrors read the pure-exp tiles
            for m in range(8):
                EW[MUL1[m]].tensor_mul(est[:, m, :], est[:, m, :], em_t[1][m][:])
        hist.append(cur)
        if len(hist) >= 3:
            finish_unit(hist[-3])
            if (hist[-3][1], hist[-3][2]) == (0, 7):
                proj_q = [(lambda mt=mt: proj_fill(0, mt)) for mt in range(8)]
                proj_q.reverse()

    for u_state in hist[-2:]:
        for m in range(8):
            emit_pv(u_state, m)
        finish_unit(u_state)
        while proj_q:
            proj_q.pop()()
    for g in sorted(sched):
        sched.pop(g)()

    # ---------------- branch-1 projection (tail) ----------------
    for mt in range(8):
        proj_fill(1, mt)


def _hilo(a):
    """Split f32 array into fp8_e4m3 hi + lo residual."""
    hi = a.astype(ml_dtypes.float8_e4m3)
    lo = (a - hi.astype(np.float32)).astype(ml_dtypes.float8_e4m3)
    return hi, lo


def _pack_pairs(a):
    """[1024, cols] -> [4, 128, 2, cols] chunk-pair layout, flattened to
    [2048?, cols] rows ((g*128+p)*2+i)."""
    ch = a.reshape(4, 2, 128, -1).transpose(0, 2, 1, 3)  # g, p, i, cols
    return np.ascontiguousarray(ch.reshape(4 * 128 * 2, -1))


def _prep_inputs(x, attn_mask, qkv_w, proj_w, proj_b):
    """8 per-core input maps: core (b, hh) = batch b, head-half hh."""
    bf = ml_dtypes.bfloat16
    q_w, k_w, v_w = qkv_w[0:C], qkv_w[C:2 * C], qkv_w[2 * C:3 * C]
    s = float(D ** (-0.5))
    em0 = np.ascontiguousarray(np.exp(attn_mask[0, 0]).T.astype(bf))
    em1 = np.ascontiguousarray(np.exp(attn_mask[1, 0]).T.astype(bf))
    # scale folding: q' = 64*s*q, k' = 32*k, v' = sqrt(2048*s)*v so that
    # q'k' = 2048*s*qk and v'v' = 2048*s*vv (GAMMA=1/2048 undone in exp),
    # while keeping the fp8 hi/lo weights in e4m3's normal range.
    # lam_v*sqrt(s) = 16 exactly, compensated in pwT.
    lam_q, lam_k = 64.0, 32.0
    lam_v = float(np.sqrt(2048.0 * s))
    in_maps = []
    xp = {}
    for core in range(8):
        b, hh = core // 2, core % 2
        sl = slice(hh * NHALF, (hh + 1) * NHALF)
        if b not in xp:
            xT = np.ascontiguousarray(x[b].T.astype(np.float32))
            xh, xl = _hilo(xT)
            xp[b] = np.vstack([_pack_pairs(xh), _pack_pairs(xl)])
        wcat = np.hstack([(q_w[sl] * (s * lam_q)).T, (k_w[sl] * lam_k).T,
                          (v_w[sl] * lam_v).T]).astype(np.float32)
        wh, wl = _hilo(wcat)
        w8 = np.vstack([_pack_pairs(wh), _pack_pairs(wl)])
        # lam_p=256 keeps pw in e4m3's normal range; host divides it out
        pwTf = (proj_w[:, sl] * (256.0 / 16.0)).T.astype(np.float32)
        ph, pl = _hilo(pwTf)
        pw8c = np.vstack([
            np.ascontiguousarray(h_.reshape(2, 2, 128, C).transpose(0, 2, 1, 3)
                                 .reshape(2 * 256, C))
            for h_ in (ph, pl)])
        in_maps.append({
            "x8": xp[b], "w8": w8, "em0": em0, "em1": em1, "pw8": pw8c,
        })
    return in_maps


def _run(inputs, trace=False, **kw):
    global _nc_cache
    from concourse.bass_utils import run_bass_kernel_spmd
    if _nc_cache is None:
        _nc_cache = _build()
    in_maps = _prep_inputs(**inputs)
    res = run_bass_kernel_spmd(_nc_cache, in_maps, core_ids=list(range(8)),
                               trace=trace, **kw)
    pb = np.asarray(inputs["proj_b"], dtype=np.float32)
    outs = []
    for br in range(2):
        nm = f"out{br}"
        ys = []
        for b in range(B):
            p0 = np.asarray(res.results[2 * b][nm], dtype=np.float32)
            p1 = np.asarray(res.results[2 * b + 1][nm], dtype=np.float32)
            ys.append((p0 + p1).T / 256.0 + pb)
        outs.append(np.stack(ys))
    x_ori, x_v = outs[0], outs[1]
    return (x_v, x_ori), res


def kernel(x, attn_mask, qkv_w, proj_w, proj_b):
    (x_v, x_ori), _ = _run(dict(x=np.asarray(x), attn_mask=np.asarray(attn_mask),
                                qkv_w=np.asarray(qkv_w), proj_w=np.asarray(proj_w),
                                proj_b=np.asarray(proj_b)))
    return (x_v, x_ori)


# revision 42
# speedup vs baseline: 1.1479x; 1.0034x over previous
"""Head-parallel dual-branch attention kernel for one TRN2 chip (8 cores).

Sharding: core (b, hh) = batch b (0-3) x head-half hh (0-1).  Each core
computes BOTH branches for its 8 heads and emits per-branch yT-partials
(proj row-sharded over channels, bf16); the host sums the two head-half
partials, rescales, and adds the bias during unshard.  Zero device comm.

Key structure (per core, SPMD-uniform):
  A1: qk [1024, N] = [q|k]-half weights^T @ x^T.  Both operands are split
      host-side into fp8-e4m3 hi + lo residual pairs; 3 cross terms
      (hi*hi + lo*hi + hi*lo) per 256-row chunk-pair via DoubleRow
      matmuls = 0.75x the bf16 PE cost at ~bf16 accuracy.  Power-of-two
      scale folding (lam_q=64, lam_k=32, lam_v=sqrt(2048/8)) keeps the
      tiny 0.02-scale weights inside e4m3's normal range; the exp's
      activation scale GAMMA=1/2048 removes it exactly.
  A2: vn [N, 512] = x @ v-half^T, same hi/lo trick; vT per head-pair via
      SBUF->SBUF DMA transpose ([128,128] blocks); vn copied both plain
      (for transposes) and into per-head 65-col strides with a ones
      column (PV rhs carrying the softmax denominator).
  per unit (br, h) of 16 (orders interleaved; br0 finishes early so the
      br0 projection overlaps the last units):
    S[m-block, n] = lhsT(kT|vT) x rhs(qT|vT), K=64, bf16 (br1: upper
      blocks only; symmetric part mirrored post-exp via DMA transpose -
      exp(S) is symmetric for (v,v) before masking)
    est = exp(GAMMA * S) on Act (the only engine walrus allows for
      activations); PSUM 'st' tiles rotate 3-deep so the PE never waits
      more than ~2 tiles on the exp.
    est *= em[br] (= exp(mask), bf16) on DVE/Pool (split 2V/6P)
    PV[n, 8x65] += est(m)^T x vn65(m): 65th col accumulates the denom.
    OT[n, 64] = PV[:, :64] * recip(PV[:, 64]) per head, written into
      head-pair otu tiles; ot[c, n] via DMA transpose (no PE transpose).
    ot is re-split into fp8 hi+lo pairs (DVE/Pool) for the projection.
  proj: yT_br[co, n] += pw^T @ ot with hi/lo fp8 DoubleRow (pw scaled by
      lam_p=256 for fp8 range; host divides it out).  br1's projection is
      the tail; br0's overlaps the last in-loop units.

Engine budget (cost model): PE ~122us, Act ~110us (128 exps - hardware
floor), Pool ~100us (mask muls), DVE ~66us, SP ~70us (DMA issue).
"""

import numpy as np
import ml_dtypes

import concourse.bass as bass
from concourse import bacc
import concourse.tile as tile
import concourse.mybir as mybir
from contextlib import ExitStack

B, N, C, H, D, P = 4, 1024, 1024, 16, 64, 128
HH = 8          # heads per core
NHALF = 512
BF16 = mybir.dt.bfloat16
F32 = mybir.dt.float32
FP8 = mybir.dt.float8e4
AF = mybir.ActivationFunctionType
PM = mybir.MatmulPerfMode
ALU = mybir.AluOpType

# psum scores arrive scaled by LAM = lam_q*lam_k (power-folded into the fp8
# hi/lo weights to stay in e4m3's normal range); exp undoes it via
# activation-scale / pow-base.
LAM = 2048.0
GAMMA = 1.0 / LAM

_nc_cache = None

# engine split tables (tuned against the cost-model trace)
# exp engine per (br, u*8+m); mul engine per (br, u*8+m)
# walrus: GPSIMD cannot access PSUM -> Pool only gets SBUF-side est*em muls;
# exps (psum reads) split Act/DVE, copies on Act, otu/rcp on DVE.
# walrus codegen rejects ALU pow on DVE too -> all exps via Act activation.
EXP0 = ["A"] * 8
EXP1 = ["A"] * 8
MUL0 = ["P", "P", "V", "P", "P", "P", "V", "V"]
MUL1 = ["P", "P", "P", "V", "P", "P", "P", "V"]


def _build(reps=1):
    nc = bacc.Bacc("TRN2", target_bir_lowering=False, debug=False, num_devices=8)
    x8 = nc.declare_dram_parameter("x8", [2 * 4 * 256, N], FP8, isOutput=False)
    w8 = nc.declare_dram_parameter("w8", [2 * 4 * 256, 3 * NHALF], FP8, isOutput=False)
    em0 = nc.declare_dram_parameter("em0", [N, N], BF16, isOutput=False)
    em1 = nc.declare_dram_parameter("em1", [N, N], BF16, isOutput=False)
    pw8 = nc.declare_dram_parameter("pw8", [2 * 2 * 256, C], FP8, isOutput=False)
    out0 = nc.declare_dram_parameter("out0", [C, N], BF16, isOutput=True)
    out1 = nc.declare_dram_parameter("out1", [C, N], BF16, isOutput=True)

    with tile.TileContext(nc) as tc:
        for _ in range(reps):
            with ExitStack() as ctx:
                _body(tc, ctx, x8, w8, em0, em1, pw8, out0, out1)
    nc.compile()
    return nc


def _body(tc, ctx, x8, w8, em0, em1, pw8, out0, out1):
    nc = tc.nc

    pers = ctx.enter_context(tc.tile_pool(name="pers", bufs=1))
    work = ctx.enter_context(tc.tile_pool(name="work", bufs=1))
    psum = ctx.enter_context(tc.tile_pool(name="psum", bufs=1, space="PSUM"))

    EW = {"A": nc.scalar, "V": nc.vector, "P": nc.gpsimd}

    def ew_copy(e, dst, src):
        if e == "A":
            nc.scalar.copy(dst, src)
        else:
            EW[e].tensor_copy(dst, src)

    # ---------------- persistent tiles ----------------
    # x8t[hl][g], w8t[hl][g]: fp8 hi/lo chunk-pair tiles for DoubleRow
    x8t = [[pers.tile([P, 2, N], FP8, name=f"x{hl}{g}", tag=f"x{hl}{g}")
            for g in range(4)] for hl in range(2)]
    w8t = [[pers.tile([P, 2, 3 * NHALF], FP8, name=f"w{hl}{g}", tag=f"w{hl}{g}")
            for g in range(4)] for hl in range(2)]
    qk_t = [pers.tile([P, N], BF16, name=f"qk{i}", tag=f"qk{i}") for i in range(8)]
    vd = pers.tile([P, 8, NHALF], BF16, name="vd", tag="vd")
    ones = pers.tile([P, 1], BF16, name="ones", tag="ones")
    vT = [pers.tile([P, N], BF16, name=f"vT{g}", tag=f"vT{g}") for g in range(4)]
    em_t = [[pers.tile([P, N], BF16, name=f"em{br}_{m}", tag=f"em{br}_{m}")
             for m in range(8)] for br in range(2)]
    Bt = pers.tile([P, N], F32, name="Bt", tag="Bt")
    ot_t = [pers.tile([P, N], BF16, name=f"ot{i}", tag=f"ot{i}") for i in range(8)]
    # hi/lo fp8 pair-tiles for the proj DR matmuls (t = ci chunk-pair; br)
    ot8 = [[[pers.tile([P, 2, N], FP8, name=f"ot8{hl}{br}{t}", tag=f"ot8{hl}{br}{t}")
             for t in range(2)] for br in range(2)] for hl in range(2)]
    pw_t = [[pers.tile([P, 2, C], FP8, name=f"pw{hl}{t}", tag=f"pw{hl}{t}")
             for t in range(2)] for hl in range(2)]

    def pts_tile(ui):
        return work.tile([P, 8, N], BF16, name=f"pts{ui}", tag="pts", bufs=3)

    def st_ps(nm):
        return psum.tile([P, N], F32, name=nm, tag="st", bufs=3)

    def pv_ps(nm):
        return psum.tile([P, 2, NHALF], F32, name=nm, tag="pv", bufs=1)

    # ---------------- input DMA staging ----------------
    # g0 first so A1 pair-0 can start immediately; issue on SP + helpers.
    def dma_x(hl, g, eng=nc.sync, half=None):
        src = x8[(hl * 4 + g) * 256:(hl * 4 + g + 1) * 256, :]
        sl = slice(0, NHALF) if half == 0 else (
            slice(NHALF, N) if half == 1 else slice(0, N))
        eng.dma_start(x8t[hl][g][:, :, sl],
                      src[:, sl].rearrange("(p two) n -> p two n", two=2))

    def dma_w(hl, g, eng=nc.sync, half=None):
        src = w8[(hl * 4 + g) * 256:(hl * 4 + g + 1) * 256, :]
        sl = slice(0, 768) if half == 0 else (
            slice(768, 3 * NHALF) if half == 1 else slice(0, 3 * NHALF))
        eng.dma_start(w8t[hl][g][:, :, sl],
                      src[:, sl].rearrange("(p two) n -> p two n", two=2))

    def dma_em(br, m, eng=nc.sync):
        src = em0 if br == 0 else em1
        eng.dma_start(em_t[br][m][:], src[m * P:(m + 1) * P, :])

    for g in range(4):
        dma_w(0, g, nc.sync, half=0)
        dma_x(0, g, nc.gpsimd, half=0)
        dma_w(1, g, nc.gpsimd, half=0)
        dma_x(1, g, nc.sync, half=0)
    nc.vector.memset(Bt[:], float(np.exp(GAMMA)))
    nc.vector.memset(ones[:], 1.0)
    # warm the Act exp table during startup (LoadActFuncSet is 1.3us)
    actw = pers.tile([P, 1], BF16, name="actw", tag="actw")
    nc.scalar.activation(actw[:], ones[:], AF.Exp)
    for g in range(4):
        dma_w(0, g, nc.sync, half=1)
        dma_x(0, g, nc.gpsimd, half=1)
        dma_w(1, g, nc.gpsimd, half=1)
        dma_x(1, g, nc.sync, half=1)
    for m in range(8):
        dma_em(0, m, nc.sync if m % 2 == 0 else nc.gpsimd)
        dma_em(1, m, nc.sync if m % 2 == 1 else nc.gpsimd)

    # ---------------- fill closures ----------------
    def hilo_mms(ps_half, lhs_of, rhs_of, first, last):
        """3-cross-term hi/lo fp8 DR accumulation over 4 chunk-pairs."""
        for g in range(4):
            combos = ((0, 0), (1, 0), (0, 1))
            for ci, (lh, rh) in enumerate(combos):
                nc.tensor.matmul(
                    ps_half,
                    lhsT=lhs_of(lh, g), rhs=rhs_of(rh, g),
                    start=(first and g == 0 and ci == 0),
                    stop=(last and g == 3 and ci == 2),
                    perf_mode=PM.DoubleRow,
                )

    def a1_fill(cc, nh, ceng):
        ps = st_ps(f"a1_{cc}_{nh}")
        half = ps[:, nh * NHALF:(nh + 1) * NHALF]
        hilo_mms(
            half,
            lambda hl, g: w8t[hl][g][:, :, cc * P:(cc + 1) * P],
            lambda hl, g: x8t[hl][g][:, :, nh * NHALF:(nh + 1) * NHALF],
            True, True,
        )
        ew_copy(ceng, qk_t[cc][:, nh * NHALF:(nh + 1) * NHALF], half)

    def a2_fill(m, ceng):
        ps = st_ps(f"a2_{m}")
        half = ps[:, 0:NHALF]
        hilo_mms(
            half,
            lambda hl, g: x8t[hl][g][:, :, m * P:(m + 1) * P],
            lambda hl, g: w8t[hl][g][:, :, 2 * NHALF:3 * NHALF],
            True, True,
        )
        ew_copy(ceng, vd[:, m, :], half)
        for g in range(4):
            nc.sync.dma_start(
                vT[g][:, m * P:(m + 1) * P],
                vd[:, m, g * P:(g + 1) * P],
                transpose=True,
            )

    def proj_fill(br, mt):
        ps = st_ps(f"y{br}_{mt}")
        out = out0 if br == 0 else out1
        for nh in range(2):
            sl = slice(nh * NHALF, (nh + 1) * NHALF)
            for t in range(2):
                for ci, (lh, rh) in enumerate(((0, 0), (1, 0), (0, 1))):
                    nc.tensor.matmul(
                        ps[:, sl],
                        lhsT=pw_t[lh][t][:, :, mt * P:(mt + 1) * P],
                        rhs=ot8[rh][br][t][:, :, sl],
                        start=(t == 0 and ci == 0), stop=(t == 1 and ci == 2),
                        perf_mode=PM.DoubleRow,
                    )
        y = work.tile([P, N], BF16, name="y", tag="y", bufs=2)
        for nh in range(2):
            sl = slice(nh * NHALF, (nh + 1) * NHALF)
            ew_copy("V" if br == 0 else "A", y[:, sl], ps[:, sl])
            nc.sync.dma_start(out[mt * P:(mt + 1) * P, sl], y[:, sl])

    # Deadline-paced fillers (gstep -> closure)
    sched = {}
    cp_cyc = ["V", "V", "V", "V", "V", "V", "V", "V"]
    for m in range(8):
        sched[m] = (lambda m=m, e=cp_cyc[m]: a2_fill(m, e))
    gs = {1: [9, 13, 17, 21], 2: [50, 54, 58, 62], 3: [74, 78, 82, 86]}
    for pair in (1, 2, 3):
        fills = [(cc, nh) for nh in range(2) for cc in (pair, 4 + pair)]
        for g, (cc, nh) in zip(gs[pair], fills):
            e = cp_cyc[(cc + nh) % 8]
            sched[g] = (lambda cc=cc, nh=nh, e=e: a1_fill(cc, nh, e))

    def late_dmas():
        for hl in range(2):
            for t in range(2):
                src_ = pw8[(hl * 2 + t) * 256:(hl * 2 + t + 1) * 256, :]
                nc.sync.dma_start(pw_t[hl][t][:],
                                  src_.rearrange("(p two) n -> p two n", two=2))
    sched[47] = late_dmas

    # upfront A1 for head-pair 0
    for nh in range(2):
        for cc in (0, 4):
            a1_fill(cc, nh, cp_cyc[(cc + nh) % 8])

    # ---------------- attention units ----------------
    UNITS = [(0, 0), (1, 0), (0, 1), (1, 1), (0, 2), (1, 2),
             (0, 3), (1, 3), (0, 4), (0, 5), (1, 4), (0, 6),
             (0, 7), (1, 5), (1, 6), (1, 7)]
    pts = {}

    def emit_pv(u_state, m):
        u, br, h, pv = u_state
        est = pts[u % 3]
        pv4 = pv[:, :, 0:260].rearrange("p b (j c) -> p b j c", c=65)
        for j in range(8):
            lhsT = est[:, m, j * P:(j + 1) * P]
            nc.tensor.matmul(
                pv4[:, j // 4, j % 4, 0:64],
                lhsT=lhsT, rhs=vd[:, m, 64 * h:64 * h + 64],
                start=(m == 0 and j % 4 == 0), stop=False,
            )
            nc.tensor.matmul(
                pv4[:, j // 4, j % 4, 64:65],
                lhsT=lhsT, rhs=ones[:],
                start=False, stop=(m == 7 and j % 4 == 3),
            )

    otu_pair = {}

    def finish_unit(u_state):
        u, br, h, pv = u_state
        g = br * 4 + h // 2
        pv4 = pv[:, :, 0:260].rearrange("p b (j c) -> p b j c", c=65)
        rcp = work.tile([P, 2, 4, 1], F32, name="rcp", tag="rcp", bufs=2)
        nc.vector.reciprocal(rcp[:], pv4[:, :, :, 64:65])
        if (br, h // 2) not in otu_pair:
            otu_pair[(br, h // 2)] = work.tile(
                [P, 2, 4, 2, 64], BF16, name=f"otu{u}", tag="otu", bufs=3)
        otu = otu_pair[(br, h // 2)]
        nc.vector.tensor_mul(
            otu[:, :, :, h % 2, :], pv4[:, :, :, 0:64],
            rcp[:].broadcast_to((P, 2, 4, 64)))
        if h % 2 == 1:
            of = otu[:].rearrange("p b j two c -> p (b j) (two c)")
            for j in range(8):
                nc.sync.dma_start(
                    ot_t[g][:, j * P:(j + 1) * P],
                    of[:, j, :],
                    transpose=True,
                )
            t, i = (h // 2) // 2, (h // 2) % 2
            ce = "P" if (h // 2) % 2 == 0 else "V"
            EW[ce].tensor_copy(ot8[0][br][t][:, i, :], ot_t[g][:])
            EW[ce].tensor_sub(ot8[1][br][t][:, i, :], ot_t[g][:],
                              ot8[0][br][t][:, i, :])

    proj_q = []
    gstep = 0
    hist = []
    for ui, (br, h) in enumerate(UNITS):
        if br == 0:
            kT, qT, ro = qk_t[4 + h // 2], qk_t[h // 2], (h % 2) * 64
        else:
            kT = qT = vT[h // 2]
            ro = (h % 2) * 64
        pv = pv_ps(f"pv{ui}")
        cur = (ui, br, h, pv)
        est = pts_tile(ui)
        pts[ui % 3] = est
        for m in range(8):
            if br == 0:
                ps = st_ps(f"st{ui}_{m}")
                for nh in range(2):
                    nc.tensor.matmul(
                        ps[:, nh * NHALF:(nh + 1) * NHALF],
                        lhsT=kT[ro:ro + 64, m * P:(m + 1) * P],
                        rhs=qT[ro:ro + 64, nh * NHALF:(nh + 1) * NHALF],
                        start=True, stop=True,
                    )
                e = EXP0[m]
                if e == "A":
                    nc.scalar.activation(est[:, m, :], ps[:], AF.Exp, scale=GAMMA)
                else:
                    EW[e].tensor_tensor(est[:, m, :], Bt[:], ps[:], ALU.pow)
                me = MUL0[m]
                EW[me].tensor_mul(est[:, m, :], est[:, m, :], em_t[0][m][:])
            else:
                mi = m
                ps = st_ps(f"st{ui}_{m}")
                if mi < 4:
                    nc.tensor.matmul(
                        ps[:, mi * P:NHALF],
                        lhsT=kT[ro:ro + 64, mi * P:(mi + 1) * P],
                        rhs=qT[ro:ro + 64, mi * P:NHALF],
                        start=True, stop=True,
                    )
                    nc.tensor.matmul(
                        ps[:, NHALF:N],
                        lhsT=kT[ro:ro + 64, mi * P:(mi + 1) * P],
                        rhs=qT[ro:ro + 64, NHALF:N],
                        start=True, stop=True,
                    )
                else:
                    nc.tensor.matmul(
                        ps[:, mi * P:N],
                        lhsT=kT[ro:ro + 64, mi * P:(mi + 1) * P],
                        rhs=qT[ro:ro + 64, mi * P:N],
                        start=True, stop=True,
                    )
                e = EXP1[m]
                if e == "A":
                    nc.scalar.activation(est[:, mi, mi * P:N], ps[:, mi * P:N],
                                         AF.Exp, scale=GAMMA)
                else:
                    EW[e].tensor_tensor(est[:, mi, mi * P:N], Bt[:, mi * P:N],
                                        ps[:, mi * P:N], ALU.pow)
                for k in range(mi):
                    nc.sync.dma_start(
                        est[:, mi, k * P:(k + 1) * P],
                        est[:, k, mi * P:(mi + 1) * P],
                        transpose=True,
                    )
            if len(hist) >= 2:
                emit_pv(hist[-2], m)
            if gstep in sched:
                sched.pop(gstep)()
            if proj_q:
                proj_q.pop()()
            gstep += 1
        if br == 1:
            # est*em only after all mirrors read the pure-exp tiles
            for m in range(8):
                EW[MUL1[m]].tensor_mul(est[:, m, :], est[:, m, :], em_t[1][m][:])
        hist.append(cur)
        if len(hist) >= 3:
            finish_unit(hist[-3])
            if (hist[-3][1], hist[-3][2]) == (0, 7):
                proj_q = [(lambda mt=mt: proj_fill(0, mt)) for mt in range(8)]
                proj_q.reverse()

    for u_state in hist[-2:]:
        for m in range(8):
            emit_pv(u_state, m)
        finish_unit(u_state)
        while proj_q:
            proj_q.pop()()
    for g in sorted(sched):
        sched.pop(g)()

    # ---------------- branch-1 projection (tail) ----------------
    for mt in range(8):
        proj_fill(1, mt)


def _hilo(a):
    """Split f32 array into fp8_e4m3 hi + lo residual."""
    hi = a.astype(ml_dtypes.float8_e4m3)
    lo = (a - hi.astype(np.float32)).astype(ml_dtypes.float8_e4m3)
    return hi, lo


def _pack_pairs(a):
    """[1024, cols] -> [4, 128, 2, cols] chunk-pair layout, flattened to
    [2048?, cols] rows ((g*128+p)*2+i)."""
    ch = a.reshape(4, 2, 128, -1).transpose(0, 2, 1, 3)  # g, p, i, cols
    return np.ascontiguousarray(ch.reshape(4 * 128 * 2, -1))


def _prep_inputs(x, attn_mask, qkv_w, proj_w, proj_b):
    """8 per-core input maps: core (b, hh) = batch b, head-half hh."""
    bf = ml_dtypes.bfloat16
    q_w, k_w, v_w = qkv_w[0:C], qkv_w[C:2 * C], qkv_w[2 * C:3 * C]
    s = float(D ** (-0.5))
    em0 = np.ascontiguousarray(np.exp(attn_mask[0, 0]).T.astype(bf))
    em1 = np.ascontiguousarray(np.exp(attn_mask[1, 0]).T.astype(bf))
    # scale folding: q' = 64*s*q, k' = 32*k, v' = sqrt(2048*s)*v so that
    # q'k' = 2048*s*qk and v'v' = 2048*s*vv (GAMMA=1/2048 undone in exp),
    # while keeping the fp8 hi/lo weights in e4m3's normal range.
    # lam_v*sqrt(s) = 16 exactly, compensated in pwT.
    lam_q, lam_k = 64.0, 32.0
    lam_v = float(np.sqrt(2048.0 * s))
    in_maps = []
    xp = {}
    for core in range(8):
        b, hh = core // 2, core % 2
        sl = slice(hh * NHALF, (hh + 1) * NHALF)
        if b not in xp:
            xT = np.ascontiguousarray(x[b].T.astype(np.float32))
            xh, xl = _hilo(xT)
            xp[b] = np.vstack([_pack_pairs(xh), _pack_pairs(xl)])
        wcat = np.hstack([(q_w[sl] * (s * lam_q)).T, (k_w[sl] * lam_k).T,
                          (v_w[sl] * lam_v).T]).astype(np.float32)
        wh, wl = _hilo(wcat)
        w8 = np.vstack([_pack_pairs(wh), _pack_pairs(wl)])
        # lam_p=256 keeps pw in e4m3's normal range; host divides it out
        pwTf = (proj_w[:, sl] * (256.0 / 16.0)).T.astype(np.float32)
        ph, pl = _hilo(pwTf)
        pw8c = np.vstack([
            np.ascontiguousarray(h_.reshape(2, 2, 128, C).transpose(0, 2, 1, 3)
                                 .reshape(2 * 256, C))
            for h_ in (ph, pl)])
        in_maps.append({
            "x8": xp[b], "w8": w8, "em0": em0, "em1": em1, "pw8": pw8c,
        })
    return in_maps


def _run(inputs, trace=False, **kw):
    global _nc_cache
    from concourse.bass_utils import run_bass_kernel_spmd
    if _nc_cache is None:
        _nc_cache = _build()
    in_maps = _prep_inputs(**inputs)
    res = run_bass_kernel_spmd(_nc_cache, in_maps, core_ids=list(range(8)),
                               trace=trace, **kw)
    pb = np.asarray(inputs["proj_b"], dtype=np.float32)
    outs = []
    for br in range(2):
        nm = f"out{br}"
        ys = []
        for b in range(B):
            p0 = np.asarray(res.results[2 * b][nm], dtype=np.float32)
            p1 = np.asarray(res.results[2 * b + 1][nm], dtype=np.float32)
            ys.append((p0 + p1).T / 256.0 + pb)
        outs.append(np.stack(ys))
    x_ori, x_v = outs[0], outs[1]
    return (x_v, x_ori), res


def kernel(x, attn_mask, qkv_w, proj_w, proj_b):
    (x_v, x_ori), _ = _run(dict(x=np.asarray(x), attn_mask=np.asarray(attn_mask),
                                qkv_w=np.asarray(qkv_w), proj_w=np.asarray(proj_w),
                                proj_b=np.asarray(proj_b)))
    return (x_v, x_ori)


# revision 49
# speedup vs baseline: 1.1739x; 1.0227x over previous
"""Head-parallel dual-branch attention kernel for one TRN2 chip (8 cores).

Sharding: core (b, hh) = batch b (0-3) x head-half hh (0-1).  Each core
computes BOTH branches for its 8 heads and emits per-branch yT-partials
(proj row-sharded over channels, f32); the host sums the two head-half
partials and adds the bias during unshard.  Zero device comm.

Key structure (per core, SPMD-uniform):
  A1: qk [1024, N] = [q|k]-half weights^T @ x^T, via fp8 hi+lo residual
      pairs and DoubleRow matmuls (3 cross terms per 256-chunk = 0.75x
      the bf16 PE cost at ~bf16 accuracy).
  A2: vn [N, 8*65] = x @ v-half^T (+ ones col per head) same hi/lo trick;
      vT per head-pair via SBUF->SBUF DMA transpose of vn (bf16).
  per unit (br, h) of 16 (br0 units first, then br1):
    S[m-block, n] = lhsT(kT|vT) x rhs(qT|vT), K=64, bf16   (br1: upper
      blocks only; symmetric part mirrored in est domain via DMA
      transpose - est is symmetric for (v,v) before masking)
    est = exp(S) on Act (activation, scale=1) or DVE/Pool
      (tensor_tensor pow: e_tile ** S) - exp work is split across all
      three elementwise engines.
    est *= em[br] (exp of mask, DVE/Pool)
    PV[n, 8x65] += est(m)^T x vn(m) (65th col = ones -> denominator)
    OT[n, 512] = PV[:, :64] * recip(PV[:, 64])  per head
    ot[c, n] via DMA transpose of OT (no PE transpose)
  proj: yT_br[co, n] += pwT^T @ ot, bf16; psum -> DRAM f32 directly.

Host adds proj bias and sums head-half partials in f32.
"""

import numpy as np
import ml_dtypes

import concourse.bass as bass
from concourse import bacc
import concourse.tile as tile
import concourse.mybir as mybir
from contextlib import ExitStack

B, N, C, H, D, P = 4, 1024, 1024, 16, 64, 128
HH = 8          # heads per core
NHALF = 512
BF16 = mybir.dt.bfloat16
F32 = mybir.dt.float32
FP8 = mybir.dt.float8e4
AF = mybir.ActivationFunctionType
PM = mybir.MatmulPerfMode
ALU = mybir.AluOpType

# psum scores arrive scaled by LAM = lam_q*lam_k (power-folded into the fp8
# hi/lo weights to stay in e4m3's normal range); exp undoes it via
# activation-scale / pow-base.
LAM = 2048.0
GAMMA = 1.0 / LAM

_nc_cache = None

# engine split tables (tuned against the cost-model trace)
# exp engine per (br, u*8+m); mul engine per (br, u*8+m)
# walrus: GPSIMD cannot access PSUM -> Pool only gets SBUF-side est*em muls;
# exps (psum reads) split Act/DVE, copies on Act, otu/rcp on DVE.
# walrus codegen rejects ALU pow on DVE too -> all exps via Act activation.
EXP0 = ["A"] * 8
EXP1 = ["A"] * 8
MUL0 = ["P", "P", "P", "V", "P", "P", "P", "V"]
MUL1 = ["P", "P", "P", "V", "P", "P", "P", "V"]


def _build(reps=1):
    nc = bacc.Bacc("TRN2", target_bir_lowering=False, debug=False, num_devices=8)
    x8 = nc.declare_dram_parameter("x8", [2 * 4 * 256, N], FP8, isOutput=False)
    w8 = nc.declare_dram_parameter("w8", [2 * 4 * 256, 3 * NHALF], FP8, isOutput=False)
    em0 = nc.declare_dram_parameter("em0", [N, N], BF16, isOutput=False)
    em1 = nc.declare_dram_parameter("em1", [N, N], BF16, isOutput=False)
    pw8 = nc.declare_dram_parameter("pw8", [2 * 2 * 256, C], FP8, isOutput=False)
    out0 = nc.declare_dram_parameter("out0", [C, N], BF16, isOutput=True)
    out1 = nc.declare_dram_parameter("out1", [C, N], BF16, isOutput=True)

    with tile.TileContext(nc) as tc:
        for _ in range(reps):
            with ExitStack() as ctx:
                _body(tc, ctx, x8, w8, em0, em1, pw8, out0, out1)
    nc.compile()
    return nc


def _body(tc, ctx, x8, w8, em0, em1, pw8, out0, out1):
    nc = tc.nc

    pers = ctx.enter_context(tc.tile_pool(name="pers", bufs=1))
    work = ctx.enter_context(tc.tile_pool(name="work", bufs=1))
    psum = ctx.enter_context(tc.tile_pool(name="psum", bufs=1, space="PSUM"))

    EW = {"A": nc.scalar, "V": nc.vector, "P": nc.gpsimd}

    def ew_copy(e, dst, src):
        if e == "A":
            nc.scalar.copy(dst, src)
        else:
            EW[e].tensor_copy(dst, src)

    # ---------------- persistent tiles ----------------
    # x8t[hl][g], w8t[hl][g]: fp8 hi/lo chunk-pair tiles for DoubleRow
    x8t = [[pers.tile([P, 2, N], FP8, name=f"x{hl}{g}", tag=f"x{hl}{g}")
            for g in range(4)] for hl in range(2)]
    w8t = [[pers.tile([P, 2, 3 * NHALF], FP8, name=f"w{hl}{g}", tag=f"w{hl}{g}")
            for g in range(4)] for hl in range(2)]
    qk_t = [pers.tile([P, N], BF16, name=f"qk{i}", tag=f"qk{i}") for i in range(8)]
    vd = pers.tile([P, 8, NHALF], BF16, name="vd", tag="vd")
    ones = pers.tile([P, 1], BF16, name="ones", tag="ones")
    vT = [pers.tile([P, N], BF16, name=f"vT{g}", tag=f"vT{g}") for g in range(4)]
    em_t = [[pers.tile([P, N], BF16, name=f"em{br}_{m}", tag=f"em{br}_{m}")
             for m in range(8)] for br in range(2)]
    Bt = pers.tile([P, N], F32, name="Bt", tag="Bt")
    ot_t = [pers.tile([P, N], BF16, name=f"ot{i}", tag=f"ot{i}") for i in range(8)]
    # hi/lo fp8 pair-tiles for the proj DR matmuls (t = ci chunk-pair; br)
    ot8 = [[[pers.tile([P, 2, N], FP8, name=f"ot8{hl}{br}{t}", tag=f"ot8{hl}{br}{t}")
             for t in range(2)] for br in range(2)] for hl in range(2)]
    pw_t = [[pers.tile([P, 2, C], FP8, name=f"pw{hl}{t}", tag=f"pw{hl}{t}")
             for t in range(2)] for hl in range(2)]

    def pts_tile(ui):
        return work.tile([P, 8, N], BF16, name=f"pts{ui}", tag="pts", bufs=3)

    def st_ps(nm):
        return psum.tile([P, N], F32, name=nm, tag="st", bufs=3)

    def pv_ps(nm):
        return psum.tile([P, 2, NHALF], F32, name=nm, tag="pv", bufs=1)

    # ---------------- input DMA staging ----------------
    # g0 first so A1 pair-0 can start immediately; issue on SP + helpers.
    def dma_x(hl, g, eng=nc.sync, half=None):
        src = x8[(hl * 4 + g) * 256:(hl * 4 + g + 1) * 256, :]
        sl = slice(0, NHALF) if half == 0 else (
            slice(NHALF, N) if half == 1 else slice(0, N))
        eng.dma_start(x8t[hl][g][:, :, sl],
                      src[:, sl].rearrange("(p two) n -> p two n", two=2))

    def dma_w(hl, g, eng=nc.sync, half=None):
        src = w8[(hl * 4 + g) * 256:(hl * 4 + g + 1) * 256, :]
        sl = slice(0, 768) if half == 0 else (
            slice(768, 3 * NHALF) if half == 1 else slice(0, 3 * NHALF))
        eng.dma_start(w8t[hl][g][:, :, sl],
                      src[:, sl].rearrange("(p two) n -> p two n", two=2))

    def dma_em(br, m, eng=nc.sync):
        src = em0 if br == 0 else em1
        eng.dma_start(em_t[br][m][:], src[m * P:(m + 1) * P, :])

    for g in range(4):
        dma_w(0, g, nc.sync, half=0)
        dma_x(0, g, nc.gpsimd, half=0)
        dma_w(1, g, nc.gpsimd, half=0)
        dma_x(1, g, nc.sync, half=0)
    nc.vector.memset(Bt[:], float(np.exp(GAMMA)))
    nc.vector.memset(ones[:], 1.0)
    # warm the Act exp table during startup (LoadActFuncSet is 1.3us)
    actw = pers.tile([P, 1], BF16, name="actw", tag="actw")
    nc.scalar.activation(actw[:], ones[:], AF.Exp)
    for g in range(4):
        dma_w(0, g, nc.sync, half=1)
        dma_x(0, g, nc.gpsimd, half=1)
        dma_w(1, g, nc.gpsimd, half=1)
        dma_x(1, g, nc.sync, half=1)
    for m in range(8):
        dma_em(0, m, nc.sync if m % 2 == 0 else nc.gpsimd)
        dma_em(1, m, nc.sync if m % 2 == 1 else nc.gpsimd)

    # ---------------- fill closures ----------------
    def hilo_mms(ps_half, lhs_of, rhs_of, first, last):
        """3-cross-term hi/lo fp8 DR accumulation over 4 chunk-pairs."""
        for g in range(4):
            combos = ((0, 0), (1, 0), (0, 1))
            for ci, (lh, rh) in enumerate(combos):
                nc.tensor.matmul(
                    ps_half,
                    lhsT=lhs_of(lh, g), rhs=rhs_of(rh, g),
                    start=(first and g == 0 and ci == 0),
                    stop=(last and g == 3 and ci == 2),
                    perf_mode=PM.DoubleRow,
                )

    def a1_fill(cc, nh, ceng):
        ps = st_ps(f"a1_{cc}_{nh}")
        half = ps[:, nh * NHALF:(nh + 1) * NHALF]
        hilo_mms(
            half,
            lambda hl, g: w8t[hl][g][:, :, cc * P:(cc + 1) * P],
            lambda hl, g: x8t[hl][g][:, :, nh * NHALF:(nh + 1) * NHALF],
            True, True,
        )
        ew_copy(ceng, qk_t[cc][:, nh * NHALF:(nh + 1) * NHALF], half)

    def a2_fill(m, ceng):
        ps = st_ps(f"a2_{m}")
        half = ps[:, 0:NHALF]
        hilo_mms(
            half,
            lambda hl, g: x8t[hl][g][:, :, m * P:(m + 1) * P],
            lambda hl, g: w8t[hl][g][:, :, 2 * NHALF:3 * NHALF],
            True, True,
        )
        ew_copy(ceng, vd[:, m, :], half)
        for g in range(4):
            nc.sync.dma_start(
                vT[g][:, m * P:(m + 1) * P],
                vd[:, m, g * P:(g + 1) * P],
                transpose=True,
            )

    def proj_fill(br, mt):
        ps = st_ps(f"y{br}_{mt}")
        out = out0 if br == 0 else out1
        for nh in range(2):
            sl = slice(nh * NHALF, (nh + 1) * NHALF)
            for t in range(2):
                for ci, (lh, rh) in enumerate(((0, 0), (1, 0), (0, 1))):
                    nc.tensor.matmul(
                        ps[:, sl],
                        lhsT=pw_t[lh][t][:, :, mt * P:(mt + 1) * P],
                        rhs=ot8[rh][br][t][:, :, sl],
                        start=(t == 0 and ci == 0), stop=(t == 1 and ci == 2),
                        perf_mode=PM.DoubleRow,
                    )
        y = work.tile([P, N], BF16, name="y", tag="y", bufs=2)
        for nh in range(2):
            sl = slice(nh * NHALF, (nh + 1) * NHALF)
            ew_copy("V" if br == 0 else "A", y[:, sl], ps[:, sl])
            nc.sync.dma_start(out[mt * P:(mt + 1) * P, sl], y[:, sl])

    # Deadline-paced fillers (gstep -> closure)
    sched = {}
    cp_cyc = ["V", "V", "V", "V", "V", "V", "V", "V"]
    for m in range(8):
        sched[m] = (lambda m=m, e=cp_cyc[m]: a2_fill(m, e))
    gs = {1: [9, 13, 17, 21], 2: [50, 54, 58, 62], 3: [74, 78, 82, 86]}
    for pair in (1, 2, 3):
        fills = [(cc, nh) for nh in range(2) for cc in (pair, 4 + pair)]
        for g, (cc, nh) in zip(gs[pair], fills):
            e = cp_cyc[(cc + nh) % 8]
            sched[g] = (lambda cc=cc, nh=nh, e=e: a1_fill(cc, nh, e))

    def late_dmas():
        for hl in range(2):
            for t in range(2):
                src_ = pw8[(hl * 2 + t) * 256:(hl * 2 + t + 1) * 256, :]
                nc.sync.dma_start(pw_t[hl][t][:],
                                  src_.rearrange("(p two) n -> p two n", two=2))
    sched[67] = late_dmas

    # upfront A1 for head-pair 0
    for nh in range(2):
        for cc in (0, 4):
            a1_fill(cc, nh, cp_cyc[(cc + nh) % 8])

    # ---------------- attention units ----------------
    UNITS = [(0, 0), (1, 0), (0, 1), (1, 1), (0, 2), (1, 2),
             (0, 3), (1, 3), (0, 4), (0, 5), (1, 4), (0, 6),
             (0, 7), (1, 5), (1, 6), (1, 7)]
    pts = {}

    def emit_pv(u_state, m):
        u, br, h, pv = u_state
        est = pts[u % 3]
        pv4 = pv[:, :, 0:260].rearrange("p b (j c) -> p b j c", c=65)
        for j in range(8):
            lhsT = est[:, m, j * P:(j + 1) * P]
            nc.tensor.matmul(
                pv4[:, j // 4, j % 4, 0:64],
                lhsT=lhsT, rhs=vd[:, m, 64 * h:64 * h + 64],
                start=(m == 0 and j % 4 == 0), stop=False,
            )
            nc.tensor.matmul(
                pv4[:, j // 4, j % 4, 64:65],
                lhsT=lhsT, rhs=ones[:],
                start=False, stop=(m == 7 and j % 4 == 3),
            )

    otu_pair = {}

    def finish_unit(u_state):
        u, br, h, pv = u_state
        g = br * 4 + h // 2
        pv4 = pv[:, :, 0:260].rearrange("p b (j c) -> p b j c", c=65)
        rcp = work.tile([P, 2, 4, 1], F32, name="rcp", tag="rcp", bufs=2)
        nc.vector.reciprocal(rcp[:], pv4[:, :, :, 64:65])
        if (br, h // 2) not in otu_pair:
            otu_pair[(br, h // 2)] = work.tile(
                [P, 2, 4, 2, 64], BF16, name=f"otu{u}", tag="otu", bufs=3)
        otu = otu_pair[(br, h // 2)]
        nc.vector.tensor_mul(
            otu[:, :, :, h % 2, :], pv4[:, :, :, 0:64],
            rcp[:].broadcast_to((P, 2, 4, 64)))
        if h % 2 == 1:
            of = otu[:].rearrange("p b j two c -> p (b j) (two c)")
            for j in range(8):
                nc.sync.dma_start(
                    ot_t[g][:, j * P:(j + 1) * P],
                    of[:, j, :],
                    transpose=True,
                )
            t, i = (h // 2) // 2, (h // 2) % 2
            ce = "P" if (h // 2) % 2 == 0 else "V"
            EW[ce].tensor_copy(ot8[0][br][t][:, i, :], ot_t[g][:])
            EW[ce].tensor_sub(ot8[1][br][t][:, i, :], ot_t[g][:],
                              ot8[0][br][t][:, i, :])

    proj_q = []
    gstep = 0
    hist = []
    for ui, (br, h) in enumerate(UNITS):
        if br == 0:
            kT, qT, ro = qk_t[4 + h // 2], qk_t[h // 2], (h % 2) * 64
        else:
            kT = qT = vT[h // 2]
            ro = (h % 2) * 64
        pv = pv_ps(f"pv{ui}")
        cur = (ui, br, h, pv)
        est = pts_tile(ui)
        pts[ui % 3] = est
        for m in range(8):
            if br == 0:
                ps = st_ps(f"st{ui}_{m}")
                for nh in range(2):
                    nc.tensor.matmul(
                        ps[:, nh * NHALF:(nh + 1) * NHALF],
                        lhsT=kT[ro:ro + 64, m * P:(m + 1) * P],
                        rhs=qT[ro:ro + 64, nh * NHALF:(nh + 1) * NHALF],
                        start=True, stop=True,
                    )
                e = EXP0[m]
                if e == "A":
                    nc.scalar.activation(est[:, m, :], ps[:], AF.Exp, scale=GAMMA)
                else:
                    EW[e].tensor_tensor(est[:, m, :], Bt[:], ps[:], ALU.pow)
                me = MUL0[m]
                EW[me].tensor_mul(est[:, m, :], est[:, m, :], em_t[0][m][:])
            else:
                mi = m
                ps = st_ps(f"st{ui}_{m}")
                if mi < 4:
                    nc.tensor.matmul(
                        ps[:, mi * P:NHALF],
                        lhsT=kT[ro:ro + 64, mi * P:(mi + 1) * P],
                        rhs=qT[ro:ro + 64, mi * P:NHALF],
                        start=True, stop=True,
                    )
                    nc.tensor.matmul(
                        ps[:, NHALF:N],
                        lhsT=kT[ro:ro + 64, mi * P:(mi + 1) * P],
                        rhs=qT[ro:ro + 64, NHALF:N],
                        start=True, stop=True,
                    )
                else:
                    nc.tensor.matmul(
                        ps[:, mi * P:N],
                        lhsT=kT[ro:ro + 64, mi * P:(mi + 1) * P],
                        rhs=qT[ro:ro + 64, mi * P:N],
                        start=True, stop=True,
                    )
                e = EXP1[m]
                if e == "A":
                    nc.scalar.activation(est[:, mi, mi * P:N], ps[:, mi * P:N],
                                         AF.Exp, scale=GAMMA)
                else:
                    EW[e].tensor_tensor(est[:, mi, mi * P:N], Bt[:, mi * P:N],
                                        ps[:, mi * P:N], ALU.pow)
                for k in range(mi):
                    nc.sync.dma_start(
                        est[:, mi, k * P:(k + 1) * P],
                        est[:, k, mi * P:(mi + 1) * P],
                        transpose=True,
                    )
            if len(hist) >= 2:
                emit_pv(hist[-2], m)
            if gstep in sched:
                sched.pop(gstep)()
            if proj_q and gstep % 2 == 0:
                proj_q.pop()()
            gstep += 1
        if br == 1:
            # est*em only after all mirrors read the pure-exp tiles
            for m in range(8):
                EW[MUL1[m]].tensor_mul(est[:, m, :], est[:, m, :], em_t[1][m][:])
        hist.append(cur)
        if len(hist) >= 3:
            finish_unit(hist[-3])
            if (hist[-3][1], hist[-3][2]) == (0, 7):
                proj_q = [(lambda mt=mt: proj_fill(0, mt)) for mt in range(8)]
                proj_q.reverse()

    for u_state in hist[-2:]:
        for m in range(8):
            emit_pv(u_state, m)
        finish_unit(u_state)
        while proj_q:
            proj_q.pop()()
    for g in sorted(sched):
        sched.pop(g)()

    # ---------------- branch-1 projection (tail) ----------------
    for mt in range(8):
        proj_fill(1, mt)


def _hilo(a):
    """Split f32 array into fp8_e4m3 hi + lo residual."""
    hi = a.astype(ml_dtypes.float8_e4m3)
    lo = (a - hi.astype(np.float32)).astype(ml_dtypes.float8_e4m3)
    return hi, lo


def _pack_pairs(a):
    """[1024, cols] -> [4, 128, 2, cols] chunk-pair layout, flattened to
    [2048?, cols] rows ((g*128+p)*2+i)."""
    ch = a.reshape(4, 2, 128, -1).transpose(0, 2, 1, 3)  # g, p, i, cols
    return np.ascontiguousarray(ch.reshape(4 * 128 * 2, -1))


def _prep_inputs(x, attn_mask, qkv_w, proj_w, proj_b):
    """8 per-core input maps: core (b, hh) = batch b, head-half hh."""
    bf = ml_dtypes.bfloat16
    q_w, k_w, v_w = qkv_w[0:C], qkv_w[C:2 * C], qkv_w[2 * C:3 * C]
    s = float(D ** (-0.5))
    em0 = np.ascontiguousarray(np.exp(attn_mask[0, 0]).T.astype(bf))
    em1 = np.ascontiguousarray(np.exp(attn_mask[1, 0]).T.astype(bf))
    # scale folding: q' = 64*s*q, k' = 32*k, v' = sqrt(2048*s)*v so that
    # q'k' = 2048*s*qk and v'v' = 2048*s*vv (GAMMA=1/2048 undone in exp),
    # while keeping the fp8 hi/lo weights in e4m3's normal range.
    # lam_v*sqrt(s) = 16 exactly, compensated in pwT.
    lam_q, lam_k = 64.0, 32.0
    lam_v = float(np.sqrt(2048.0 * s))
    in_maps = []
    xp = {}
    for core in range(8):
        b, hh = core // 2, core % 2
        sl = slice(hh * NHALF, (hh + 1) * NHALF)
        if b not in xp:
            xT = np.ascontiguousarray(x[b].T.astype(np.float32))
            xh, xl = _hilo(xT)
            xp[b] = np.vstack([_pack_pairs(xh), _pack_pairs(xl)])
        wcat = np.hstack([(q_w[sl] * (s * lam_q)).T, (k_w[sl] * lam_k).T,
                          (v_w[sl] * lam_v).T]).astype(np.float32)
        wh, wl = _hilo(wcat)
        w8 = np.vstack([_pack_pairs(wh), _pack_pairs(wl)])
        # lam_p=256 keeps pw in e4m3's normal range; host divides it out
        pwTf = (proj_w[:, sl] * (256.0 / 16.0)).T.astype(np.float32)
        ph, pl = _hilo(pwTf)
        pw8c = np.vstack([
            np.ascontiguousarray(h_.reshape(2, 2, 128, C).transpose(0, 2, 1, 3)
                                 .reshape(2 * 256, C))
            for h_ in (ph, pl)])
        in_maps.append({
            "x8": xp[b], "w8": w8, "em0": em0, "em1": em1, "pw8": pw8c,
        })
    return in_maps


def _run(inputs, trace=False, **kw):
    global _nc_cache
    from concourse.bass_utils import run_bass_kernel_spmd
    if _nc_cache is None:
        _nc_cache = _build()
    in_maps = _prep_inputs(**inputs)
    res = run_bass_kernel_spmd(_nc_cache, in_maps, core_ids=list(range(8)),
                               trace=trace, **kw)
    pb = np.asarray(inputs["proj_b"], dtype=np.float32)
    outs = []
    for br in range(2):
        nm = f"out{br}"
        ys = []
        for b in range(B):
            p0 = np.asarray(res.results[2 * b][nm], dtype=np.float32)
            p1 = np.asarray(res.results[2 * b + 1][nm], dtype=np.float32)
            ys.append((p0 + p1).T / 256.0 + pb)
        outs.append(np.stack(ys))
    x_ori, x_v = outs[0], outs[1]
    return (x_v, x_ori), res


def kernel(x, attn_mask, qkv_w, proj_w, proj_b):
    (x_v, x_ori), _ = _run(dict(x=np.asarray(x), attn_mask=np.asarray(attn_mask),
                                qkv_w=np.asarray(qkv_w), proj_w=np.asarray(proj_w),
                                proj_b=np.asarray(proj_b)))
    return (x_v, x_ori)
